# revision 1
# baseline (speedup 1.0000x reference)
"""Two-layer GAT on 8 trn2 NeuronCores (SPMD Bass kernel).

Sharding: nodes are permuted into 392 degree-balanced bins of 128 ("blocks"),
8 cores x 49 blocks.  Edges are assigned to the destination node's block.
Per block, edges are split into two streams by which half of the (permuted)
node table their source lives in (dma_gather indices are int16), padded to
a fixed tile count with sentinel edges whose exp(logit) is exactly 0.

Device program per core (identical; per-core data differs):
  Phase A : h_ext = x @ [W1 | W1@a_src | W1@a_dst]  (fp32r matmuls over the
            full graph; every core computes the full table).  h stored bf16 +
            alpha_src fp32 in 768B rows across two half-tables; alpha_dst in
            a 4-nodes-per-256B-row table (gathered by dst>>2).
  L1      : per block: dma_gather source rows + alpha_dst rows, per-edge
            p = exp(leakyrelu(as+ad)), segment softmax + weighted scatter-add
            via one-hot matmuls accumulated in PSUM, divide, +b1, ELU; then
            h2 = out1 @ [W2 | W2@a_src2 | W2@a_dst2] for the block (layer-2
            projection fused here).
  AllGather h2 shards (8.4MB) -> full h2 table, reshape to gather tables.
  L2      : same edge machinery at 256B rows, then log_softmax per block.
"""
import numpy as np

N = 50000
IN_DIM = 256
HID = 64
HEADS = 4
OUT_DIM = 40
E = 800000
NEG = 0.2

NC = 8
BLOCKS_PER_CORE = 49
NBLK = NC * BLOCKS_PER_CORE            # 392
NODES_PER_CORE = BLOCKS_PER_CORE * 128  # 6272
NTOT = NBLK * 128                       # 50176
HALF = NTOT // 2                        # 25088
SENT = HALF                             # sentinel row index in each half-table
ROW1 = 192    # f32 words per L1 table row (768B): h bf16[256] | as1 f32[4] | pad
ROW2 = 64     # f32 words per L2 table row (256B): h2 f32[40] | as2 | ad2 | pad
ADROW = 64    # f32 words per ad-table row (256B): 4 nodes x [ad1 f32[4] | ad2 | pad 11]
SENT_AS = -60000.0


def _prep(inputs):
    x = np.asarray(inputs["x"], dtype=np.float32)
    ei = np.asarray(inputs["edge_index"])
    W1 = np.asarray(inputs["W1"], dtype=np.float32)
    as1 = np.asarray(inputs["att_src1"], dtype=np.float32)
    ad1 = np.asarray(inputs["att_dst1"], dtype=np.float32)
    b1 = np.asarray(inputs["b1"], dtype=np.float32)
    W2 = np.asarray(inputs["W2"], dtype=np.float32)
    as2 = np.asarray(inputs["att_src2"], dtype=np.float32)
    ad2 = np.asarray(inputs["att_dst2"], dtype=np.float32)
    b2 = np.asarray(inputs["b2"], dtype=np.float32)

    src = np.concatenate([ei[0], np.arange(N, dtype=ei.dtype)]).astype(np.int64)
    dst = np.concatenate([ei[1], np.arange(N, dtype=ei.dtype)]).astype(np.int64)
    ne = src.shape[0]

    # --- node -> (bin, slot) assignment: snake over 392 bins by in-degree desc
    deg = np.bincount(dst, minlength=N)
    order = np.argsort(-deg, kind="stable")
    pos = np.empty(N, dtype=np.int64)  # node -> permuted position
    nfull = N // NBLK                  # full snake rounds (127)
    for r in range(nfull + 1):
        lo = r * NBLK
        hi = min(lo + NBLK, N)
        if lo >= hi:
            break
        nodes = order[lo:hi]
        bins = np.arange(hi - lo)
        if r % 2 == 1:
            bins = NBLK - 1 - bins
        pos[nodes] = bins * 128 + r
    # slots: each bin's slot index = round r (0..127); bins hit by < 128 rounds
    # leave trailing dummy slots.

    spos = pos[src]
    dpos = pos[dst]
    core = dpos // NODES_PER_CORE
    blk = (dpos % NODES_PER_CORE) // 128
    slot = dpos % 128
    gbin = core * BLOCKS_PER_CORE + blk  # == dpos // 128
    is_lo = spos < HALF
    srow = np.where(is_lo, spos, spos - HALF)

    # per (gbin, stream) counts
    cnt_lo = np.bincount(gbin[is_lo], minlength=NBLK)
    cnt_hi = np.bincount(gbin[~is_lo], minlength=NBLK)
    T_LO = int(np.ceil(cnt_lo.max() / 128))
    T_HI = int(np.ceil(cnt_hi.max() / 128))
    T = T_LO + T_HI

    # canvases [NBLK, T*128] position-major (pos t -> tile t//128... t = tile*128+p)
    def build_canvas(mask, ntiles, cnt):
        n_pad = ntiles * 128
        c_src = np.full((NBLK, n_pad), SENT, dtype=np.int64)
        c_slot = np.zeros((NBLK, n_pad), dtype=np.int64)
        c_dpos = np.empty((NBLK, n_pad), dtype=np.int64)
        c_dpos[:] = (np.arange(NBLK) * 128)[:, None]  # pad edges -> block slot-0 node
        g = gbin[mask]
        o = np.argsort(g, kind="stable")
        g = g[o]
        starts = np.zeros(NBLK + 1, dtype=np.int64)
        np.cumsum(np.bincount(g, minlength=NBLK), out=starts[1:])
        within = np.arange(g.shape[0]) - starts[g]
        flat = g * n_pad + within
        c_src.reshape(-1)[flat] = srow[mask][o]
        c_slot.reshape(-1)[flat] = slot[mask][o]
        c_dpos.reshape(-1)[flat] = dpos[mask][o]
        return c_src, c_slot, c_dpos

    clo_src, clo_slot, clo_dpos = build_canvas(is_lo, T_LO, cnt_lo)
    chi_src, chi_slot, chi_dpos = build_canvas(~is_lo, T_HI, cnt_hi)
    c_slot = np.concatenate([clo_slot.reshape(NBLK, T_LO, 128),
                             chi_slot.reshape(NBLK, T_HI, 128)], axis=1)
    c_dpos = np.concatenate([clo_dpos.reshape(NBLK, T_LO, 128),
                             chi_dpos.reshape(NBLK, T_HI, 128)], axis=1)

    def wrap_idx(canvas, ntiles):
        # [NBLK, ntiles*128] int -> [NBLK, 128, ntiles*8] int16 (16-wrap, replicated)
        n = ntiles * 128
        w = canvas.reshape(NBLK, n // 16, 16).transpose(0, 2, 1).astype(np.int16)
        return np.tile(w, (1, 8, 1)).copy()  # [NBLK, 128, n/16]

    idx_lo = wrap_idx(clo_src, T_LO)
    idx_hi = wrap_idx(chi_src, T_HI)
    idx_ad = wrap_idx((c_dpos >> 2).reshape(NBLK, T * 128), T)

    # dstloc [NBLK, 128, T]: value at (p, tile) = slot of edge tile*128+p
    dstloc = c_slot.transpose(0, 2, 1).astype(np.float32).copy()
    # d4rep [NBLK, 128, T*4]: (dpos & 3) repeated 4x along s
    d4 = (c_dpos & 3).transpose(0, 2, 1).astype(np.float32)  # [NBLK,128,T]
    d4rep = np.repeat(d4, 4, axis=2).copy()
    iota4rep = np.tile(np.tile(np.arange(4, dtype=np.float32), T), (128, 1)).copy()

    # weights
    W1e = np.zeros((IN_DIM, 264), dtype=np.float32)
    W1e[:, :256] = W1
    for h in range(HEADS):
        W1e[:, 256 + h] = W1[:, h * HID:(h + 1) * HID] @ as1[h]
        W1e[:, 260 + h] = W1[:, h * HID:(h + 1) * HID] @ ad1[h]
    W2e = np.zeros((IN_DIM, 42), dtype=np.float32)
    W2e[:, :40] = W2
    W2e[:, 40] = W2 @ as2[0]
    W2e[:, 41] = W2 @ ad2[0]

    xT = np.zeros((IN_DIM, NTOT), dtype=np.float32)
    xT[:, pos] = x.T

    b1r = np.tile(b1[None, :], (128, 1)).astype(np.float32).copy()
    b2r = np.tile(b2[None, :], (128, 1)).astype(np.float32).copy()
    iota128 = np.tile(np.arange(128, dtype=np.float32)[None, :], (128, 1)).copy()
    ident = np.eye(128, dtype=np.float32)
    sent1 = np.zeros((1, ROW1), dtype=np.float32)
    sent1[0, 128:132] = SENT_AS
    sent2 = np.zeros((1, ROW2), dtype=np.float32)
    sent2[0, 40] = SENT_AS

    shared = dict(xT=xT, W1e=W1e, W2e=W2e, b1r=b1r, b2r=b2r,
                  iota128=iota128, iota4rep=iota4rep, ident=ident,
                  sent1=sent1, sent2=sent2)
    percore = []
    for c in range(NC):
        s = slice(c * BLOCKS_PER_CORE, (c + 1) * BLOCKS_PER_CORE)
        percore.append(dict(idx_lo=idx_lo[s], idx_hi=idx_hi[s], idx_ad=idx_ad[s],
                            dstloc=dstloc[s], d4rep=d4rep[s]))
    return shared, percore, (T_LO, T_HI), pos


def _build(T_LO, T_HI, phases="full"):
    reps = 1
    if phases.startswith("fullx"):
        reps = int(phases[5:])
        phases = "full"
    import concourse.bass as bass
    import concourse.bacc as bacc
    import concourse.mybir as mybir
    import concourse.tile as tile

    f32 = mybir.dt.float32
    f32r = mybir.dt.float32r
    bf16 = mybir.dt.bfloat16
    i16 = mybir.dt.int16
    Alu = mybir.AluOpType
    Act = mybir.ActivationFunctionType
    T = T_LO + T_HI

    nc = bacc.Bacc("TRN2", target_bir_lowering=False, debug=False, num_devices=NC)

    xT = nc.dram_tensor("xT", [IN_DIM, NTOT], f32r, kind="ExternalInput")
    W1e_d = nc.dram_tensor("W1e", [IN_DIM, 264], f32r, kind="ExternalInput")
    W2e_d = nc.dram_tensor("W2e", [IN_DIM, 42], f32, kind="ExternalInput")
    b1r_d = nc.dram_tensor("b1r", [128, 256], f32, kind="ExternalInput")
    b2r_d = nc.dram_tensor("b2r", [128, OUT_DIM], f32, kind="ExternalInput")
    iota128_d = nc.dram_tensor("iota128", [128, 128], f32, kind="ExternalInput")
    iota4_d = nc.dram_tensor("iota4rep", [128, T * 4], f32, kind="ExternalInput")
    ident_d = nc.dram_tensor("ident", [128, 128], f32, kind="ExternalInput")
    sent1_d = nc.dram_tensor("sent1", [1, ROW1], f32, kind="ExternalInput")
    sent2_d = nc.dram_tensor("sent2", [1, ROW2], f32, kind="ExternalInput")
    idx_lo_d = nc.dram_tensor("idx_lo", [BLOCKS_PER_CORE, 128, T_LO * 8], i16, kind="ExternalInput")
    idx_hi_d = nc.dram_tensor("idx_hi", [BLOCKS_PER_CORE, 128, T_HI * 8], i16, kind="ExternalInput")
    idx_ad_d = nc.dram_tensor("idx_ad", [BLOCKS_PER_CORE, 128, T * 8], i16, kind="ExternalInput")
    dstloc_d = nc.dram_tensor("dstloc", [BLOCKS_PER_CORE, 128, T], f32, kind="ExternalInput")
    d4rep_d = nc.dram_tensor("d4rep", [BLOCKS_PER_CORE, 128, T * 4], f32, kind="ExternalInput")
    out_d = nc.dram_tensor("out", [NODES_PER_CORE, OUT_DIM], f32, kind="ExternalOutput")

    def ap(view, dims, extra_off=0):
        return bass.AP(view.tensor, view.offset + extra_off, [list(view.ap[0])] + dims)

    def gather_chunks(g_tile, tab_ap, idx_tile, ntiles, elem):
        for c0 in range(0, ntiles, 8):
            cn = min(8, ntiles - c0)
            nc.gpsimd.dma_gather(
                out_ap=g_tile[:, c0:c0 + cn, :], in_ap=tab_ap,
                idxs_ap=idx_tile[:, c0 * 8:(c0 + cn) * 8],
                num_idxs=cn * 128, num_idxs_reg=cn * 128, elem_size=elem)

    with tile.TileContext(nc) as tc:
        with tc.tile_pool(name="dram", bufs=1, space="DRAM") as dram, \
             tc.tile_pool(name="const", bufs=1) as cpool:
            tabL1_lo = dram.tile([HALF + 1, ROW1], f32)
            tabL1_hi = dram.tile([HALF + 1, ROW1], f32)
            adtab = dram.tile([NTOT // 4, ADROW], f32)
            tabL2_lo = dram.tile([HALF + 1, ROW2], f32)
            tabL2_hi = dram.tile([HALF + 1, ROW2], f32)
            h2_shard = dram.tile([NODES_PER_CORE, 42], f32)
            h2_full = dram.tile([NTOT, 42], f32, addr_space="Shared")

            w1e0 = cpool.tile([128, 264], f32r)
            w1e1 = cpool.tile([128, 264], f32r)
            nc.sync.dma_start(out=w1e0[:], in_=W1e_d[0:128, :])
            nc.sync.dma_start(out=w1e1[:], in_=W1e_d[128:256, :])
            w2e0 = cpool.tile([128, 42], f32)
            w2e1 = cpool.tile([128, 42], f32)
            nc.sync.dma_start(out=w2e0[:], in_=W2e_d[0:128, :])
            nc.sync.dma_start(out=w2e1[:], in_=W2e_d[128:256, :])
            b1r_t = cpool.tile([128, 256], f32)
            b2r_t = cpool.tile([128, OUT_DIM], f32)
            nc.sync.dma_start(out=b1r_t[:], in_=b1r_d[:])
            nc.sync.dma_start(out=b2r_t[:], in_=b2r_d[:])
            io128 = cpool.tile([128, 128], f32)
            nc.sync.dma_start(out=io128[:], in_=iota128_d[:])
            io4 = cpool.tile([128, T * 4], f32)
            nc.sync.dma_start(out=io4[:], in_=iota4_d[:])
            id_t = cpool.tile([128, 128], f32)
            nc.sync.dma_start(out=id_t[:], in_=ident_d[:])
            s1_t = cpool.tile([1, ROW1], f32)
            nc.sync.dma_start(out=s1_t[:], in_=sent1_d[:])
            nc.sync.dma_start(out=tabL1_lo[SENT:SENT + 1, :], in_=s1_t[:])
            nc.sync.dma_start(out=tabL1_hi[SENT:SENT + 1, :], in_=s1_t[:])
            s2_t = cpool.tile([1, ROW2], f32)
            nc.sync.dma_start(out=s2_t[:], in_=sent2_d[:])
            nc.sync.dma_start(out=tabL2_lo[SENT:SENT + 1, :], in_=s2_t[:])
            nc.sync.dma_start(out=tabL2_hi[SENT:SENT + 1, :], in_=s2_t[:])

            for _rep in range(reps):
                # ---------------- Phase A ----------------
                with tc.tile_pool(name="pa_x", bufs=4) as pax, \
                     tc.tile_pool(name="pa_ps", bufs=4, space="PSUM") as paps, \
                     tc.tile_pool(name="pa_row", bufs=4) as parow, \
                     tc.tile_pool(name="pa_ad", bufs=4) as paad:
                    for nt in range(NBLK):
                        xa0 = pax.tile([128, 128], f32r, tag="xa0")
                        xa1 = pax.tile([128, 128], f32r, tag="xa1")
                        nc.sync.dma_start(out=xa0[:], in_=xT[0:128, nt * 128:(nt + 1) * 128])
                        nc.sync.dma_start(out=xa1[:], in_=xT[128:256, nt * 128:(nt + 1) * 128])
                        ps = paps.tile([128, 264], f32)
                        nc.tensor.matmul(out=ps[:], lhsT=xa0[:], rhs=w1e0[:], start=True, stop=False)
                        nc.tensor.matmul(out=ps[:], lhsT=xa1[:], rhs=w1e1[:], start=False, stop=True)
                        row = parow.tile([128, 132], f32)
                        nc.vector.tensor_copy(out=row[:, 0:128].bitcast(bf16), in_=ps[:, 0:256])
                        nc.vector.tensor_copy(out=row[:, 128:132], in_=ps[:, 256:260])
                        adsb = paad.tile([128, 4], f32)
                        nc.vector.tensor_copy(out=adsb[:], in_=ps[:, 260:264])
                        tab = tabL1_lo if nt < NBLK // 2 else tabL1_hi
                        r0 = (nt * 128) % HALF
                        nc.sync.dma_start(out=tab[r0:r0 + 128, 0:132], in_=row[:])
                        adv = adtab[nt * 32:(nt + 1) * 32, :].rearrange("r (s f) -> r s f", s=4)[:, :, 0:4]
                        nc.sync.dma_start(out=adv, in_=adsb[:])

                # ---------------- L1 edge phase (+ fused layer-2 projection) ----
                if phases == "A":
                    return nc
                l1sub, l1n = "full", BLOCKS_PER_CORE
                if phases.startswith("L1:"):
                    _, l1sub, _n = phases.split(":")
                    l1n = int(_n)
                with tc.tile_pool(name="g1", bufs=2) as g1p, \
                     tc.tile_pool(name="gad", bufs=2) as gadp, \
                     tc.tile_pool(name="gidx", bufs=2) as gip, \
                     tc.tile_pool(name="meta", bufs=2) as metap, \
                     tc.tile_pool(name="scr", bufs=2) as scrp, \
                     tc.tile_pool(name="sS", bufs=4) as sSp, \
                     tc.tile_pool(name="post", bufs=2) as postp, \
                     tc.tile_pool(name="l1ps", bufs=2, space="PSUM") as l1ps, \
                     tc.tile_pool(name="tps", bufs=2, space="PSUM") as tps, \
                     tc.tile_pool(name="a2ps", bufs=2, space="PSUM") as a2ps:
                    for b in range(l1n):
                        il = gip.tile([128, T_LO * 8], i16, tag="il")
                        ih = gip.tile([128, T_HI * 8], i16, tag="ih")
                        ia = gip.tile([128, T * 8], i16, tag="ia")
                        nc.sync.dma_start(out=il[:], in_=idx_lo_d[b])
                        nc.sync.dma_start(out=ih[:], in_=idx_hi_d[b])
                        nc.sync.dma_start(out=ia[:], in_=idx_ad_d[b])
                        dl = metap.tile([128, T], f32, tag="dl")
                        d4 = metap.tile([128, T * 4], f32, tag="d4")
                        nc.sync.dma_start(out=dl[:], in_=dstloc_d[b])
                        nc.sync.dma_start(out=d4[:], in_=d4rep_d[b])

                        glo = g1p.tile([128, T_LO, ROW1], f32, tag="glo")
                        ghi = g1p.tile([128, T_HI, ROW1], f32, tag="ghi")
                        gad = gadp.tile([128, T, ADROW], f32, tag="gad")
                        gather_chunks(glo, tabL1_lo[:], il, T_LO, ROW1)
                        gather_chunks(ghi, tabL1_hi[:], ih, T_HI, ROW1)
                        gather_chunks(gad, adtab[:], ia, T, ADROW)

                        if l1sub == "g":
                            continue
                        # one-hot over (dpos & 3): oh4 [128, T, 4]
                        oh4 = scrp.tile([128, T * 4], f32, tag="oh4")
                        nc.vector.tensor_tensor(out=oh4[:], in0=io4[:, 0:T * 4], in1=d4[:],
                                                op=Alu.is_equal)
                        # ad1 select: prod [128, T, 4f, 4s] -> reduce s
                        prod = scrp.tile([128, T * 16], f32, tag="prod")
                        gadv = gad[:]
                        nc.vector.tensor_tensor(
                            out=prod[:].rearrange("p (t f s) -> p t f s", f=4, s=4),
                            in0=ap(gadv, [[ADROW, T], [1, 4], [16, 4]]),
                            in1=ap(oh4[:], [[4, T], [0, 4], [1, 4]]),
                            op=Alu.mult)
                        ade = scrp.tile([128, T * 4], f32, tag="ade")
                        nc.vector.tensor_reduce(
                            out=ade[:].rearrange("p (t f) -> p t f", f=4),
                            in_=prod[:].rearrange("p (t f s) -> p t f s", f=4, s=4),
                            op=Alu.add, axis=mybir.AxisListType.X)
                        # logits = as + ad  (split by stream), lrelu, exp
                        pe = scrp.tile([128, T * 4], f32, tag="pe")
                        pev = pe[:].rearrange("p (t f) -> p t f", f=4)
                        adev = ade[:].rearrange("p (t f) -> p t f", f=4)
                        nc.vector.tensor_tensor(out=pev[:, 0:T_LO, :], in0=adev[:, 0:T_LO, :],
                                                in1=glo[:, :, 128:132], op=Alu.add)
                        nc.vector.tensor_tensor(out=pev[:, T_LO:T, :], in0=adev[:, T_LO:T, :],
                                                in1=ghi[:, :, 128:132], op=Alu.add)
                        u = scrp.tile([128, T * 4], f32, tag="u")
                        nc.vector.tensor_scalar_mul(out=u[:], in0=pe[:], scalar1=NEG)
                        nc.vector.tensor_tensor(out=pe[:], in0=pe[:], in1=u[:], op=Alu.max)
                        nc.scalar.activation(out=pe[:], in_=pe[:], func=Act.Exp)
                        # p -> bf16 into rows at word 128 (256th bf16 slot)
                        nc.vector.tensor_copy(out=glo[:, :, 128:130].bitcast(bf16),
                                              in_=pev[:, 0:T_LO, :])
                        nc.vector.tensor_copy(out=ghi[:, :, 128:130].bitcast(bf16),
                                              in_=pev[:, T_LO:T, :])
                        # h *= p (per head), bf16
                        for g_t, tlo, nT in ((glo, 0, T_LO), (ghi, T_LO, T_HI)):
                            hb = g_t[:].bitcast(bf16)  # [128, nT, 384]
                            nc.vector.tensor_tensor(
                                out=ap(hb, [[384, nT], [64, 4], [1, 64]]),
                                in0=ap(hb, [[384, nT], [64, 4], [1, 64]]),
                                in1=ap(hb, [[384, nT], [1, 4], [0, 64]], extra_off=256),
                                op=Alu.mult)

                        if l1sub == "p":
                            continue
                        psb = l1ps.tile([128, 260], f32)
                        for t in range(T):
                            S = sSp.tile([128, 128], bf16, tag="S")
                            nc.vector.tensor_scalar(out=S[:], in0=io128[:], scalar1=dl[:, t:t + 1],
                                                    scalar2=None, op0=Alu.is_equal)
                            g_t, tt = (glo, t) if t < T_LO else (ghi, t - T_LO)
                            rhs = g_t[:, tt, 0:130].bitcast(bf16)
                            nc.tensor.matmul(out=psb[:], lhsT=S[:], rhs=rhs,
                                             start=(t == 0), stop=(t == T - 1))
                        # divide + bias + ELU
                        dn = postp.tile([128, 4], f32, tag="dn")
                        nc.vector.tensor_scalar_add(out=dn[:], in0=psb[:, 256:260], scalar1=1e-16)
                        rcp = postp.tile([128, 4], f32, tag="rcp")
                        nc.vector.reciprocal(out=rcp[:], in_=dn[:])
                        o1 = postp.tile([128, 256], f32, tag="o1")
                        o1v = o1[:].rearrange("p (h c) -> p h c", h=4)
                        nc.vector.tensor_tensor(out=o1v, in0=psb[:, 0:256].rearrange("p (h c) -> p h c", h=4),
                                                in1=ap(rcp[:], [[1, 4], [0, 64]]), op=Alu.mult)
                        nc.vector.tensor_tensor(out=o1[:], in0=o1[:], in1=b1r_t[:], op=Alu.add)
                        em = postp.tile([128, 256], f32, tag="em")
                        nc.vector.tensor_scalar_min(out=em[:], in0=o1[:], scalar1=0.0)
                        nc.scalar.activation(out=em[:], in_=em[:], func=Act.Exp)
                        nc.vector.tensor_scalar_max(out=o1[:], in0=o1[:], scalar1=0.0)
                        nc.vector.tensor_tensor(out=o1[:], in0=o1[:], in1=em[:], op=Alu.add)
                        nc.vector.tensor_scalar_add(out=o1[:], in0=o1[:], scalar1=-1.0)
                        if l1sub == "m":
                            continue
                        # layer-2 projection for this block: h2 = o1 @ W2e
                        ps2 = a2ps.tile([128, 42], f32)
                        for c_i, w2c in ((0, w2e0), (1, w2e1)):
                            pst = tps.tile([128, 128], f32)
                            nc.tensor.transpose(out=pst[:], in_=o1[:, c_i * 128:(c_i + 1) * 128],
                                                identity=id_t[:])
                            tsb = postp.tile([128, 128], f32, tag=f"tsb{c_i}")
                            nc.vector.tensor_copy(out=tsb[:], in_=pst[:])
                            nc.tensor.matmul(out=ps2[:], lhsT=tsb[:], rhs=w2c[:],
                                             start=(c_i == 0), stop=(c_i == 1))
                        h2row = postp.tile([128, 42], f32, tag="h2row")
                        nc.vector.tensor_copy(out=h2row[:], in_=ps2[:])
                        nc.sync.dma_start(out=h2_shard[b * 128:(b + 1) * 128, :], in_=h2row[:])

                # ---------------- AllGather + L2 table build ----------------
                if phases == "A1" or phases.startswith("L1:"):
                    return nc
                nc.gpsimd.collective_compute(
                    "AllGather", mybir.AluOpType.bypass,
                    replica_groups=[list(range(NC))],
                    ins=[h2_shard[:]], outs=[h2_full[:]])
                CH = 12544
                for c0 in range(0, HALF, CH):
                    nc.gpsimd.dma_start(out=tabL2_lo[c0:c0 + CH, 0:42],
                                        in_=h2_full[c0:c0 + CH, :])
                    nc.gpsimd.dma_start(out=tabL2_hi[c0:c0 + CH, 0:42],
                                        in_=h2_full[HALF + c0:HALF + c0 + CH, :])
                adv2 = adtab[:].rearrange("r (s f) -> r s f", s=4)[:, :, 4:5]
                for c0 in range(0, NTOT, CH):
                    nc.gpsimd.dma_start(out=adv2[c0 // 4:(c0 + CH) // 4],
                                        in_=h2_full[c0:c0 + CH, 41:42])

                # ---------------- L2 edge phase ----------------
                if phases == "A1C":
                    return nc
                with tc.tile_pool(name="g2", bufs=2) as g2p, \
                     tc.tile_pool(name="gad2", bufs=2) as gad2p, \
                     tc.tile_pool(name="gidx2", bufs=2) as gip2, \
                     tc.tile_pool(name="meta2", bufs=2) as metap2, \
                     tc.tile_pool(name="scr2", bufs=2) as scrp2, \
                     tc.tile_pool(name="sS2", bufs=4) as sSp2, \
                     tc.tile_pool(name="post2", bufs=2) as postp2, \
                     tc.tile_pool(name="l2ps", bufs=2, space="PSUM") as l2ps:
                    for b in range(BLOCKS_PER_CORE):
                        il = gip2.tile([128, T_LO * 8], i16, tag="il2")
                        ih = gip2.tile([128, T_HI * 8], i16, tag="ih2")
                        ia = gip2.tile([128, T * 8], i16, tag="ia2")
                        nc.sync.dma_start(out=il[:], in_=idx_lo_d[b])
                        nc.sync.dma_start(out=ih[:], in_=idx_hi_d[b])
                        nc.sync.dma_start(out=ia[:], in_=idx_ad_d[b])
                        dl = metap2.tile([128, T], f32, tag="dl2")
                        d4 = metap2.tile([128, T * 4], f32, tag="d42")
                        nc.sync.dma_start(out=dl[:], in_=dstloc_d[b])
                        nc.sync.dma_start(out=d4[:], in_=d4rep_d[b])

                        glo = g2p.tile([128, T_LO, ROW2], f32, tag="glo2")
                        ghi = g2p.tile([128, T_HI, ROW2], f32, tag="ghi2")
                        gad = gad2p.tile([128, T, ADROW], f32, tag="gad2")
                        gather_chunks(glo, tabL2_lo[:], il, T_LO, ROW2)
                        gather_chunks(ghi, tabL2_hi[:], ih, T_HI, ROW2)
                        gather_chunks(gad, adtab[:], ia, T, ADROW)

                        oh4 = scrp2.tile([128, T * 4], f32, tag="oh42")
                        nc.vector.tensor_tensor(out=oh4[:], in0=io4[:, 0:T * 4], in1=d4[:],
                                                op=Alu.is_equal)
                        prod = scrp2.tile([128, T * 4], f32, tag="prod2")
                        nc.vector.tensor_tensor(
                            out=prod[:].rearrange("p (t f s) -> p t f s", f=1, s=4),
                            in0=ap(gad[:], [[ADROW, T], [1, 1], [16, 4]], extra_off=4),
                            in1=ap(oh4[:], [[4, T], [0, 1], [1, 4]]),
                            op=Alu.mult)
                        ade = scrp2.tile([128, T], f32, tag="ade2")
                        nc.vector.tensor_reduce(
                            out=ade[:].rearrange("p (t f) -> p t f", f=1),
                            in_=prod[:].rearrange("p (t f s) -> p t f s", f=1, s=4),
                            op=Alu.add, axis=mybir.AxisListType.X)
                        pe = scrp2.tile([128, T], f32, tag="pe2")
                        pev = pe[:].rearrange("p (t f) -> p t f", f=1)
                        adev = ade[:].rearrange("p (t f) -> p t f", f=1)
                        nc.vector.tensor_tensor(out=pev[:, 0:T_LO, :], in0=adev[:, 0:T_LO, :],
                                                in1=glo[:, :, 40:41], op=Alu.add)
                        nc.vector.tensor_tensor(out=pev[:, T_LO:T, :], in0=adev[:, T_LO:T, :],
                                                in1=ghi[:, :, 40:41], op=Alu.add)
                        u = scrp2.tile([128, T], f32, tag="u2")
                        nc.vector.tensor_scalar_mul(out=u[:], in0=pe[:], scalar1=NEG)
                        nc.vector.tensor_tensor(out=pe[:], in0=pe[:], in1=u[:], op=Alu.max)
                        nc.scalar.activation(out=pe[:], in_=pe[:], func=Act.Exp)
                        # h2 *= p2 ; write p2 into word 40
                        for g_t, tlo, nT in ((glo, 0, T_LO), (ghi, T_LO, T_HI)):
                            nc.vector.tensor_tensor(
                                out=ap(g_t[:], [[ROW2, nT], [1, 40]]),
                                in0=ap(g_t[:], [[ROW2, nT], [1, 40]]),
                                in1=ap(pe[:], [[1, nT], [0, 40]], extra_off=tlo),
                                op=Alu.mult)
                            nc.vector.tensor_copy(out=g_t[:, :, 40:41],
                                                  in_=pev[:, tlo:tlo + nT, :])
                        psb = l2ps.tile([128, 41], f32)
                        for t in range(T):
                            S = sSp2.tile([128, 128], f32, tag="S2")
                            nc.vector.tensor_scalar(out=S[:], in0=io128[:], scalar1=dl[:, t:t + 1],
                                                    scalar2=None, op0=Alu.is_equal)
                            g_t, tt = (glo, t) if t < T_LO else (ghi, t - T_LO)
                            nc.tensor.matmul(out=psb[:], lhsT=S[:], rhs=g_t[:, tt, 0:41],
                                             start=(t == 0), stop=(t == T - 1))
                        dn = postp2.tile([128, 1], f32, tag="dn2")
                        nc.vector.tensor_scalar_add(out=dn[:], in0=psb[:, 40:41], scalar1=1e-16)
                        rcp = postp2.tile([128, 1], f32, tag="rcp2")
                        nc.vector.reciprocal(out=rcp[:], in_=dn[:])
                        o2 = postp2.tile([128, OUT_DIM], f32, tag="o2")
                        nc.vector.tensor_scalar(out=o2[:], in0=psb[:, 0:40], scalar1=rcp[:, 0:1],
                                                scalar2=None, op0=Alu.mult)
                        nc.vector.tensor_tensor(out=o2[:], in0=o2[:], in1=b2r_t[:], op=Alu.add)
                        mx = postp2.tile([128, 1], f32, tag="mx")
                        nc.vector.tensor_reduce(out=mx[:], in_=o2[:], op=Alu.max,
                                                axis=mybir.AxisListType.X)
                        mxn = postp2.tile([128, 1], f32, tag="mxn")
                        nc.vector.tensor_scalar_mul(out=mxn[:], in0=mx[:], scalar1=-1.0)
                        ex = postp2.tile([128, OUT_DIM], f32, tag="ex")
                        nc.scalar.activation(out=ex[:], in_=o2[:], func=Act.Exp, bias=mxn[:, 0:1])
                        sm = postp2.tile([128, 1], f32, tag="sm")
                        nc.vector.tensor_reduce(out=sm[:], in_=ex[:], op=Alu.add,
                                                axis=mybir.AxisListType.X)
                        lns = postp2.tile([128, 1], f32, tag="lns")
                        nc.scalar.activation(out=lns[:], in_=sm[:], func=Act.Ln)
                        tot = postp2.tile([128, 1], f32, tag="tot")
                        nc.vector.tensor_tensor(out=tot[:], in0=mx[:], in1=lns[:], op=Alu.add)
                        o2f = postp2.tile([128, OUT_DIM], f32, tag="o2f")
                        nc.vector.tensor_scalar(out=o2f[:], in0=o2[:], scalar1=tot[:, 0:1],
                                                scalar2=None, op0=Alu.subtract)
                        nc.sync.dma_start(out=out_d[b * 128:(b + 1) * 128, :], in_=o2f[:])
    return nc


_CACHE = {}


LAST_EXEC_NS = -1


def kernel(**inputs):
    return _run(inputs, "full")


def _run(inputs, phases, trace=False, tmpdir=None):
    from concourse.bass_utils import run_bass_kernel_spmd
    shared, percore, (T_LO, T_HI), pos = _prep(inputs)
    key = (T_LO, T_HI, phases)
    if key not in _CACHE:
        nc = _build(T_LO, T_HI, phases)
        nc.compile()
        _CACHE[key] = nc
    nc = _CACHE[key]
    in_maps = []
    for c in range(NC):
        m = dict(shared)
        m.update(percore[c])
        in_maps.append(m)
    res = run_bass_kernel_spmd(nc, in_maps, list(range(NC)), trace=trace, tmpdir=tmpdir)
    global LAST_EXEC_NS
    if res.exec_time_ns is not None:
        LAST_EXEC_NS = res.exec_time_ns
    full = np.concatenate([res.results[c]["out"] for c in range(NC)], axis=0)
    return np.ascontiguousarray(full[pos]).astype(np.float32)



# revision 2
# speedup vs baseline: 1.7796x; 1.7796x over previous
"""Two-layer GAT on 8 trn2 NeuronCores (SPMD Bass kernel), v2.

Profiling on trn2 showed the bottleneck is gpsimd descriptor generation for
dma_gather (~8 ns per gathered row, serialized on the Pool engine), with the
per-tile DVE one-hot builds (~900 ns each) second.  v2 therefore minimizes
gathered ROW COUNT and ships the one-hot matrices from the host:

- Nodes are permuted into 392 degree-balanced bins of 128 (8 cores x 49
  blocks); edges are assigned to the destination's block and split into two
  streams by source half (int16 gather indices).  A 2-D greedy pass balances
  per-bin (lo, hi) stream loads to minimize tile count T.
- Per (block, tile): scatter one-hot S[p,q]=(slot(p)==q) and its transpose
  S_T are shipped as fp8 host data (exact 0/1; fp8 lhsT x bf16 rhs matmul
  verified exact on hw).  Pad slots gather row 0 and carry zero one-hot
  columns, so they contribute nothing.
- dst-side attention coefficients are never gathered per edge: ad1 per edge
  comes from T matmuls (lhsT=S_T, rhs=block's own ad1 [128,4]); ad2 per edge
  is computed the same way in L1 (S_T still in SBUF) and stashed in SBUF for
  L2.  The block's own ad1 is fetched with a single 128-row dma_gather from
  a 2-nodes-per-row table + parity select (SPMD-uniform addressing).
- Phase A (x @ [W1 | W1 a_s | W1 a_d]) runs in bf16 (fp32r measured 534
  ns/matmul; bf16 ~110), x shipped bf16 (halves DMA).
- Layer-2 table is one bf16 AllGather output [NTOT, 128] (256B rows); lo/hi
  gathers address row-range views; no local rebuild pass.
"""
import numpy as np
import ml_dtypes

N = 50000
IN_DIM = 256
HID = 64
HEADS = 4
OUT_DIM = 40
E = 800000
NEG = 0.2

NC = 8
BLOCKS_PER_CORE = 49
NBLK = NC * BLOCKS_PER_CORE            # 392
NODES_PER_CORE = BLOCKS_PER_CORE * 128  # 6272
NTOT = NBLK * 128                       # 50176
HALF = NTOT // 2                        # 25088
ROW1 = 192    # f32 words per L1 table row (768B): h bf16[256] | as1 f32[4] | pad
ROW2 = 128    # bf16 words per L2 table row (256B): h2 bf16[40] | as2 bf16 | pad


def _prep(inputs):
    x = np.asarray(inputs["x"], dtype=np.float32)
    ei = np.asarray(inputs["edge_index"])
    W1 = np.asarray(inputs["W1"], dtype=np.float32)
    as1 = np.asarray(inputs["att_src1"], dtype=np.float32)
    ad1 = np.asarray(inputs["att_dst1"], dtype=np.float32)
    b1 = np.asarray(inputs["b1"], dtype=np.float32)
    W2 = np.asarray(inputs["W2"], dtype=np.float32)
    as2 = np.asarray(inputs["att_src2"], dtype=np.float32)
    ad2 = np.asarray(inputs["att_dst2"], dtype=np.float32)
    b2 = np.asarray(inputs["b2"], dtype=np.float32)

    src = np.concatenate([ei[0], np.arange(N, dtype=ei.dtype)]).astype(np.int64)
    dst = np.concatenate([ei[1], np.arange(N, dtype=ei.dtype)]).astype(np.int64)

    # ---- pass 1: split nodes into lo/hi halves by degree snake (as v1) ----
    deg = np.bincount(dst, minlength=N)
    order = np.argsort(-deg, kind="stable")
    half_of = np.empty(N, dtype=np.int8)   # 0 = lo half, 1 = hi half
    nfull = N // NBLK
    bins0 = np.empty(N, dtype=np.int64)
    for r in range(nfull + 1):
        lo = r * NBLK
        hi = min(lo + NBLK, N)
        if lo >= hi:
            break
        nodes = order[lo:hi]
        b = np.arange(hi - lo)
        if r % 2 == 1:
            b = NBLK - 1 - b
        bins0[nodes] = b
    half_of[:] = (bins0 >= NBLK // 2)

    # ---- pass 2: per-node (lo, hi) in-degree, greedy 2-D balance ----
    src_half = half_of[src]
    dlo = np.bincount(dst[src_half == 0], minlength=N)
    dhi = np.bincount(dst[src_half == 1], minlength=N)
    pos = np.empty(N, dtype=np.int64)
    HB = NBLK // 2
    for h in (0, 1):
        nodes = np.where(half_of == h)[0]
        nodes = nodes[np.argsort(-(dlo[nodes] + dhi[nodes]), kind="stable")]
        load_lo = np.zeros(HB, dtype=np.int64)
        load_hi = np.zeros(HB, dtype=np.int64)
        count = np.zeros(HB, dtype=np.int64)
        slot_base = (np.arange(HB) + h * HB) * 128
        for n in nodes:
            score = np.maximum(load_lo + dlo[n], load_hi + dhi[n])
            score[count >= 128] = 1 << 60
            g = int(np.argmin(score))
            pos[n] = slot_base[g] + count[g]
            count[g] += 1
            load_lo[g] += dlo[n]
            load_hi[g] += dhi[n]

    spos = pos[src]
    dpos = pos[dst]
    gbin = dpos // 128
    slot = dpos % 128
    is_lo = spos < HALF
    srow = np.where(is_lo, spos, spos - HALF)

    cnt_lo = np.bincount(gbin[is_lo], minlength=NBLK)
    cnt_hi = np.bincount(gbin[~is_lo], minlength=NBLK)
    T_LO = int(np.ceil(cnt_lo.max() / 128))
    T_HI = int(np.ceil(cnt_hi.max() / 128))
    T = T_LO + T_HI

    def build_canvas(mask, ntiles):
        n_pad = ntiles * 128
        c_src = np.zeros((NBLK, n_pad), dtype=np.int64)     # pad -> row 0
        c_slot = np.full((NBLK, n_pad), -1, dtype=np.int64)  # pad -> -1
        g = gbin[mask]
        o = np.argsort(g, kind="stable")
        g = g[o]
        starts = np.zeros(NBLK + 1, dtype=np.int64)
        np.cumsum(np.bincount(g, minlength=NBLK), out=starts[1:])
        within = np.arange(g.shape[0]) - starts[g]
        flat = g * n_pad + within
        c_src.reshape(-1)[flat] = srow[mask][o]
        c_slot.reshape(-1)[flat] = slot[mask][o]
        return c_src, c_slot

    clo_src, clo_slot = build_canvas(is_lo, T_LO)
    chi_src, chi_slot = build_canvas(~is_lo, T_HI)
    c_slot = np.concatenate([clo_slot.reshape(NBLK, T_LO, 128),
                             chi_slot.reshape(NBLK, T_HI, 128)], axis=1)

    def wrap_idx(canvas, ntiles):
        n = ntiles * 128
        w = canvas.reshape(NBLK, n // 16, 16).transpose(0, 2, 1).astype(np.int16)
        return np.tile(w, (1, 8, 1)).copy()  # [NBLK, 128, n/16]

    idx_lo = wrap_idx(clo_src, T_LO)
    idx_hi = wrap_idx(chi_src, T_HI)

    # block-ad gather indices: 128 idx = (gbin*128 + p) >> 1, wrapped
    padr = (np.arange(NBLK)[:, None] * 128 + np.arange(128)[None, :]) >> 1
    idx_ad = wrap_idx(padr.reshape(NBLK, 128), 1)  # [NBLK, 128, 8]

    # one-hot stacks, fp8 (exact 0/1). pad slots (c_slot=-1) stay all-zero.
    valid = c_slot >= 0                        # [NBLK, T, 128]
    g_i, t_i, p_i = np.nonzero(valid)
    q_i = c_slot[valid]
    Sstk = np.zeros((NBLK, 128, T * 128), dtype=ml_dtypes.float8_e4m3fn)
    STstk = np.zeros((NBLK, 128, T * 128), dtype=ml_dtypes.float8_e4m3fn)
    one = np.float32(1.0).astype(ml_dtypes.float8_e4m3fn)
    Sstk[g_i, p_i, t_i * 128 + q_i] = one
    STstk[g_i, q_i, t_i * 128 + p_i] = one

    # weights (bf16)
    W1e = np.zeros((IN_DIM, 264), dtype=np.float32)
    W1e[:, :256] = W1
    for h in range(HEADS):
        W1e[:, 256 + h] = W1[:, h * HID:(h + 1) * HID] @ as1[h]
        W1e[:, 260 + h] = W1[:, h * HID:(h + 1) * HID] @ ad1[h]
    W2e = np.zeros((IN_DIM, 42), dtype=np.float32)
    W2e[:, :40] = W2
    W2e[:, 40] = W2 @ as2[0]
    W2e[:, 41] = W2 @ ad2[0]

    xT = np.zeros((IN_DIM, NTOT), dtype=np.float32)
    xT[:, pos] = x.T

    b1r = np.tile(b1[None, :], (128, 1)).astype(np.float32).copy()
    b2r = np.tile(b2[None, :], (128, 1)).astype(np.float32).copy()
    ident = np.eye(128, dtype=np.float32)
    parity = (np.arange(128, dtype=np.float32) % 2).reshape(128, 1).copy()

    shared = dict(xTb=xT.astype(ml_dtypes.bfloat16),
                  W1e=W1e.astype(ml_dtypes.bfloat16),
                  W2e=W2e.astype(ml_dtypes.bfloat16),
                  b1r=b1r, b2r=b2r, ident=ident, parity=parity)
    percore = []
    for c in range(NC):
        s = slice(c * BLOCKS_PER_CORE, (c + 1) * BLOCKS_PER_CORE)
        percore.append(dict(idx_lo=idx_lo[s], idx_hi=idx_hi[s],
                            idx_ad=idx_ad[s], sstk=Sstk[s], ststk=STstk[s]))
    return shared, percore, (T_LO, T_HI), pos


def _build(T_LO, T_HI, phases="full"):
    import concourse.bass as bass
    import concourse.bacc as bacc
    import concourse.mybir as mybir
    import concourse.tile as tile

    f32 = mybir.dt.float32
    bf16 = mybir.dt.bfloat16
    fp8 = mybir.dt.float8e4
    i16 = mybir.dt.int16
    Alu = mybir.AluOpType
    Act = mybir.ActivationFunctionType
    T = T_LO + T_HI

    nc = bacc.Bacc("TRN2", target_bir_lowering=False, debug=False,
                   num_devices=NC, num_swdge_queues=4)

    xTb = nc.dram_tensor("xTb", [IN_DIM, NTOT], bf16, kind="ExternalInput")
    W1e_d = nc.dram_tensor("W1e", [IN_DIM, 264], bf16, kind="ExternalInput")
    W2e_d = nc.dram_tensor("W2e", [IN_DIM, 42], bf16, kind="ExternalInput")
    b1r_d = nc.dram_tensor("b1r", [128, 256], f32, kind="ExternalInput")
    b2r_d = nc.dram_tensor("b2r", [128, OUT_DIM], f32, kind="ExternalInput")
    ident_d = nc.dram_tensor("ident", [128, 128], f32, kind="ExternalInput")
    par_d = nc.dram_tensor("parity", [128, 1], f32, kind="ExternalInput")
    idx_lo_d = nc.dram_tensor("idx_lo", [BLOCKS_PER_CORE, 128, T_LO * 8], i16, kind="ExternalInput")
    idx_hi_d = nc.dram_tensor("idx_hi", [BLOCKS_PER_CORE, 128, T_HI * 8], i16, kind="ExternalInput")
    idx_ad_d = nc.dram_tensor("idx_ad", [BLOCKS_PER_CORE, 128, 8], i16, kind="ExternalInput")
    sstk_d = nc.dram_tensor("sstk", [BLOCKS_PER_CORE, 128, T * 128], fp8, kind="ExternalInput")
    ststk_d = nc.dram_tensor("ststk", [BLOCKS_PER_CORE, 128, T * 128], fp8, kind="ExternalInput")
    out_d = nc.dram_tensor("out", [NODES_PER_CORE, OUT_DIM], f32, kind="ExternalOutput")

    def ap(view, dims, extra_off=0):
        return bass.AP(view.tensor, view.offset + extra_off, [list(view.ap[0])] + dims)

    with tile.TileContext(nc) as tc:
        with tc.tile_pool(name="dram", bufs=1, space="DRAM") as dram, \
             tc.tile_pool(name="const", bufs=1) as cpool, \
             tc.tile_pool(name="stash", bufs=1) as stash:
            tabL1_lo = dram.tile([HALF, ROW1], f32)
            tabL1_hi = dram.tile([HALF, ROW1], f32)
            blockad = dram.tile([NTOT // 2, 64], f32)
            h2shard = dram.tile([NODES_PER_CORE, ROW2], bf16)
            tabL2 = dram.tile([NTOT, ROW2], bf16, addr_space="Shared")

            w1e0 = cpool.tile([128, 264], bf16)
            w1e1 = cpool.tile([128, 264], bf16)
            nc.sync.dma_start(out=w1e0[:], in_=W1e_d[0:128, :])
            nc.sync.dma_start(out=w1e1[:], in_=W1e_d[128:256, :])
            w2e0 = cpool.tile([128, 42], bf16)
            w2e1 = cpool.tile([128, 42], bf16)
            nc.sync.dma_start(out=w2e0[:], in_=W2e_d[0:128, :])
            nc.sync.dma_start(out=w2e1[:], in_=W2e_d[128:256, :])
            b1r_t = cpool.tile([128, 256], f32)
            b2r_t = cpool.tile([128, OUT_DIM], f32)
            nc.sync.dma_start(out=b1r_t[:], in_=b1r_d[:])
            nc.sync.dma_start(out=b2r_t[:], in_=b2r_d[:])
            id_t = cpool.tile([128, 128], f32)
            nc.sync.dma_start(out=id_t[:], in_=ident_d[:])
            par_t = cpool.tile([128, 1], f32)
            nc.sync.dma_start(out=par_t[:], in_=par_d[:])
            ad2st = stash.tile([128, BLOCKS_PER_CORE * T], f32)

            # ---------------- Phase A ----------------
            with tc.tile_pool(name="pa_x", bufs=4) as pax, \
                 tc.tile_pool(name="pa_ps", bufs=4, space="PSUM") as paps, \
                 tc.tile_pool(name="pa_row", bufs=4) as parow, \
                 tc.tile_pool(name="pa_ad", bufs=4) as paad:
                for nt in range(NBLK):
                    xa0 = pax.tile([128, 128], bf16, tag="xa0")
                    xa1 = pax.tile([128, 128], bf16, tag="xa1")
                    nc.sync.dma_start(out=xa0[:], in_=xTb[0:128, nt * 128:(nt + 1) * 128])
                    nc.sync.dma_start(out=xa1[:], in_=xTb[128:256, nt * 128:(nt + 1) * 128])
                    ps = paps.tile([128, 264], f32)
                    nc.tensor.matmul(out=ps[:], lhsT=xa0[:], rhs=w1e0[:], start=True, stop=False)
                    nc.tensor.matmul(out=ps[:], lhsT=xa1[:], rhs=w1e1[:], start=False, stop=True)
                    row = parow.tile([128, 132], f32)
                    nc.vector.tensor_copy(out=row[:, 0:128].bitcast(bf16), in_=ps[:, 0:256])
                    nc.vector.tensor_copy(out=row[:, 128:132], in_=ps[:, 256:260])
                    adsb = paad.tile([128, 4], f32)
                    nc.vector.tensor_copy(out=adsb[:], in_=ps[:, 260:264])
                    tab = tabL1_lo if nt < NBLK // 2 else tabL1_hi
                    r0 = (nt * 128) % HALF
                    nc.sync.dma_start(out=tab[r0:r0 + 128, 0:132], in_=row[:])
                    adv = blockad[nt * 64:(nt + 1) * 64, :].rearrange(
                        "r (s f) -> r s f", s=16)[:, 0:2, 0:4]
                    nc.sync.dma_start(out=adv, in_=adsb[:])

            if phases == "A":
                return nc

            # ---------------- L1 edge phase (+ fused layer-2 projection) ----
            l1n = BLOCKS_PER_CORE
            if phases.startswith("L1:"):
                l1n = int(phases.split(":")[1])
            with tc.tile_pool(name="g1", bufs=2) as g1p, \
                 tc.tile_pool(name="gidx", bufs=2) as gip, \
                 tc.tile_pool(name="sstk", bufs=2) as ssp, \
                 tc.tile_pool(name="gad", bufs=2) as gadp, \
                 tc.tile_pool(name="scr", bufs=2) as scrp, \
                 tc.tile_pool(name="post", bufs=2) as postp, \
                 tc.tile_pool(name="l1ps", bufs=2, space="PSUM") as l1ps, \
                 tc.tile_pool(name="tps", bufs=2, space="PSUM") as tps, \
                 tc.tile_pool(name="a2ps", bufs=2, space="PSUM") as a2ps, \
                 tc.tile_pool(name="adps", bufs=2, space="PSUM") as adpsp:
                for b in range(l1n):
                    il = gip.tile([128, T_LO * 8], i16, tag="il")
                    ih = gip.tile([128, T_HI * 8], i16, tag="ih")
                    ia = gip.tile([128, 8], i16, tag="ia")
                    nc.sync.dma_start(out=il[:], in_=idx_lo_d[b])
                    nc.sync.dma_start(out=ih[:], in_=idx_hi_d[b])
                    nc.sync.dma_start(out=ia[:], in_=idx_ad_d[b])
                    sk = ssp.tile([128, T * 128], fp8, tag="sk")
                    stk = ssp.tile([128, T * 128], fp8, tag="stk")
                    nc.sync.dma_start(out=sk[:], in_=sstk_d[b])
                    nc.sync.dma_start(out=stk[:], in_=ststk_d[b])

                    glo = g1p.tile([128, T_LO, ROW1], f32, tag="glo")
                    ghi = g1p.tile([128, T_HI, ROW1], f32, tag="ghi")
                    qn = 0
                    for g_t, tab, idxs, nt_s in ((glo, tabL1_lo, il, T_LO),
                                                 (ghi, tabL1_hi, ih, T_HI)):
                        for c0 in range(0, nt_s, 8):
                            cn = min(8, nt_s - c0)
                            nc.gpsimd.dma_gather(
                                out_ap=g_t[:, c0:c0 + cn, :], in_ap=tab[:],
                                idxs_ap=idxs[:, c0 * 8:(c0 + cn) * 8],
                                num_idxs=cn * 128, num_idxs_reg=cn * 128,
                                elem_size=ROW1, queue_num=qn % 4)
                            qn += 1
                    gad = gadp.tile([128, 1, 64], f32, tag="gad")
                    nc.gpsimd.dma_gather(
                        out_ap=gad[:], in_ap=blockad[:], idxs_ap=ia[:],
                        num_idxs=128, num_idxs_reg=128, elem_size=64,
                        queue_num=qn % 4)

                    # block ad1 via parity select: ad = adA + par*(adB - adA)
                    dfa = scrp.tile([128, 4], f32, tag="dfa")
                    nc.vector.tensor_tensor(out=dfa[:], in0=gad[:, 0, 4:8],
                                            in1=gad[:, 0, 0:4], op=Alu.subtract)
                    nc.vector.tensor_tensor(
                        out=dfa[:], in0=dfa[:],
                        in1=ap(par_t[:], [[0, 4]]), op=Alu.mult)
                    adblk = scrp.tile([128, 4], bf16, tag="adblk")
                    nc.vector.tensor_tensor(out=adblk[:], in0=gad[:, 0, 0:4],
                                            in1=dfa[:], op=Alu.add)

                    # per-edge ad1: T matmuls lhsT=S_T fp8
                    adp = adpsp.tile([128, T * 4 + T], f32)
                    for t in range(T):
                        nc.tensor.matmul(out=adp[:, t * 4:(t + 1) * 4],
                                         lhsT=stk[:, t * 128:(t + 1) * 128],
                                         rhs=adblk[:], start=True, stop=True)

                    # logits = as + ad, lrelu, exp
                    pe = scrp.tile([128, T * 4], f32, tag="pe")
                    pev = pe[:].rearrange("p (t f) -> p t f", f=4)
                    adv_ = adp[:, 0:T * 4].rearrange("p (t f) -> p t f", f=4)
                    nc.vector.tensor_tensor(out=pev[:, 0:T_LO, :], in0=adv_[:, 0:T_LO, :],
                                            in1=glo[:, :, 128:132], op=Alu.add)
                    nc.vector.tensor_tensor(out=pev[:, T_LO:T, :], in0=adv_[:, T_LO:T, :],
                                            in1=ghi[:, :, 128:132], op=Alu.add)
                    u = scrp.tile([128, T * 4], f32, tag="u")
                    nc.vector.tensor_scalar_mul(out=u[:], in0=pe[:], scalar1=NEG)
                    nc.vector.tensor_tensor(out=pe[:], in0=pe[:], in1=u[:], op=Alu.max)
                    nc.scalar.activation(out=pe[:], in_=pe[:], func=Act.Exp)
                    # p -> bf16 into rows at word 128
                    nc.vector.tensor_copy(out=glo[:, :, 128:130].bitcast(bf16),
                                          in_=pev[:, 0:T_LO, :])
                    nc.vector.tensor_copy(out=ghi[:, :, 128:130].bitcast(bf16),
                                          in_=pev[:, T_LO:T, :])
                    # h *= p (per head), bf16
                    for g_t, nT in ((glo, T_LO), (ghi, T_HI)):
                        hb = g_t[:].bitcast(bf16)
                        nc.vector.tensor_tensor(
                            out=ap(hb, [[384, nT], [64, 4], [1, 64]]),
                            in0=ap(hb, [[384, nT], [64, 4], [1, 64]]),
                            in1=ap(hb, [[384, nT], [1, 4], [0, 64]], extra_off=256),
                            op=Alu.mult)

                    # scatter-add via one-hot matmuls
                    psb = l1ps.tile([128, 260], f32)
                    for t in range(T):
                        g_t, tt = (glo, t) if t < T_LO else (ghi, t - T_LO)
                        nc.tensor.matmul(out=psb[:],
                                         lhsT=sk[:, t * 128:(t + 1) * 128],
                                         rhs=g_t[:, tt, 0:130].bitcast(bf16),
                                         start=(t == 0), stop=(t == T - 1))
                    # divide + bias + ELU
                    dn = postp.tile([128, 4], f32, tag="dn")
                    nc.vector.tensor_scalar_add(out=dn[:], in0=psb[:, 256:260], scalar1=1e-16)
                    rcp = postp.tile([128, 4], f32, tag="rcp")
                    nc.vector.reciprocal(out=rcp[:], in_=dn[:])
                    o1 = postp.tile([128, 256], f32, tag="o1")
                    o1v = o1[:].rearrange("p (h c) -> p h c", h=4)
                    nc.vector.tensor_tensor(out=o1v, in0=psb[:, 0:256].rearrange("p (h c) -> p h c", h=4),
                                            in1=ap(rcp[:], [[1, 4], [0, 64]]), op=Alu.mult)
                    nc.vector.tensor_tensor(out=o1[:], in0=o1[:], in1=b1r_t[:], op=Alu.add)
                    em = postp.tile([128, 256], f32, tag="em")
                    nc.vector.tensor_scalar_min(out=em[:], in0=o1[:], scalar1=0.0)
                    nc.scalar.activation(out=em[:], in_=em[:], func=Act.Exp)
                    nc.vector.tensor_scalar_max(out=o1[:], in0=o1[:], scalar1=0.0)
                    nc.vector.tensor_tensor(out=o1[:], in0=o1[:], in1=em[:], op=Alu.add)
                    nc.vector.tensor_scalar_add(out=o1[:], in0=o1[:], scalar1=-1.0)
                    # layer-2 projection: h2 = o1 @ W2e (bf16)
                    ps2 = a2ps.tile([128, 42], f32)
                    for c_i, w2c in ((0, w2e0), (1, w2e1)):
                        pst = tps.tile([128, 128], f32)
                        nc.tensor.transpose(out=pst[:], in_=o1[:, c_i * 128:(c_i + 1) * 128],
                                            identity=id_t[:])
                        tsb = postp.tile([128, 128], bf16, tag=f"tsb{c_i}")
                        nc.vector.tensor_copy(out=tsb[:], in_=pst[:])
                        nc.tensor.matmul(out=ps2[:], lhsT=tsb[:], rhs=w2c[:],
                                         start=(c_i == 0), stop=(c_i == 1))
                    h2row = postp.tile([128, 41], bf16, tag="h2row")
                    nc.vector.tensor_copy(out=h2row[:], in_=ps2[:, 0:41])
                    nc.sync.dma_start(out=h2shard[b * 128:(b + 1) * 128, 0:41], in_=h2row[:])
                    # per-edge ad2 via S_T (still in SBUF), stash for L2
                    adblk2 = postp.tile([128, 1], bf16, tag="adblk2")
                    nc.vector.tensor_copy(out=adblk2[:], in_=ps2[:, 41:42])
                    for t in range(T):
                        nc.tensor.matmul(out=adp[:, T * 4 + t:T * 4 + t + 1],
                                         lhsT=stk[:, t * 128:(t + 1) * 128],
                                         rhs=adblk2[:], start=True, stop=True)
                    nc.vector.tensor_copy(out=ad2st[:, b * T:(b + 1) * T],
                                          in_=adp[:, T * 4:T * 4 + T])

            if phases == "A1" or phases.startswith("L1:"):
                return nc

            # ---------------- AllGather ----------------
            nc.gpsimd.collective_compute(
                "AllGather", mybir.AluOpType.bypass,
                replica_groups=[list(range(NC))],
                ins=[h2shard[:]], outs=[tabL2[:]])

            if phases == "A1C":
                return nc

            # ---------------- L2 edge phase ----------------
            with tc.tile_pool(name="g2", bufs=2) as g2p, \
                 tc.tile_pool(name="gidx2", bufs=2) as gip2, \
                 tc.tile_pool(name="sstk2", bufs=2) as ssp2, \
                 tc.tile_pool(name="scr2", bufs=2) as scrp2, \
                 tc.tile_pool(name="post2", bufs=2) as postp2, \
                 tc.tile_pool(name="l2ps", bufs=2, space="PSUM") as l2ps:
                for b in range(BLOCKS_PER_CORE):
                    il = gip2.tile([128, T_LO * 8], i16, tag="il2")
                    ih = gip2.tile([128, T_HI * 8], i16, tag="ih2")
                    nc.sync.dma_start(out=il[:], in_=idx_lo_d[b])
                    nc.sync.dma_start(out=ih[:], in_=idx_hi_d[b])
                    sk = ssp2.tile([128, T * 128], fp8, tag="sk2")
                    nc.sync.dma_start(out=sk[:], in_=sstk_d[b])

                    glo = g2p.tile([128, T_LO, ROW2], bf16, tag="glo2")
                    ghi = g2p.tile([128, T_HI, ROW2], bf16, tag="ghi2")
                    qn = 0
                    for g_t, r0, r1, idxs, nt_s in (
                            (glo, 0, HALF, il, T_LO),
                            (ghi, HALF, NTOT, ih, T_HI)):
                        for c0 in range(0, nt_s, 8):
                            cn = min(8, nt_s - c0)
                            nc.gpsimd.dma_gather(
                                out_ap=g_t[:, c0:c0 + cn, :],
                                in_ap=tabL2[r0:r1, :],
                                idxs_ap=idxs[:, c0 * 8:(c0 + cn) * 8],
                                num_idxs=cn * 128, num_idxs_reg=cn * 128,
                                elem_size=ROW2, queue_num=qn % 4)
                            qn += 1

                    pe = scrp2.tile([128, T], f32, tag="pe2")
                    pev = pe[:].rearrange("p (t f) -> p t f", f=1)
                    adv_ = ad2st[:, b * T:(b + 1) * T].rearrange("p (t f) -> p t f", f=1)
                    nc.vector.tensor_tensor(out=pev[:, 0:T_LO, :], in0=adv_[:, 0:T_LO, :],
                                            in1=glo[:, :, 40:41], op=Alu.add)
                    nc.vector.tensor_tensor(out=pev[:, T_LO:T, :], in0=adv_[:, T_LO:T, :],
                                            in1=ghi[:, :, 40:41], op=Alu.add)
                    u = scrp2.tile([128, T], f32, tag="u2")
                    nc.vector.tensor_scalar_mul(out=u[:], in0=pe[:], scalar1=NEG)
                    nc.vector.tensor_tensor(out=pe[:], in0=pe[:], in1=u[:], op=Alu.max)
                    nc.scalar.activation(out=pe[:], in_=pe[:], func=Act.Exp)
                    # h2 *= p2 ; write p2 into word 40
                    for g_t, tlo, nT in ((glo, 0, T_LO), (ghi, T_LO, T_HI)):
                        nc.vector.tensor_tensor(
                            out=ap(g_t[:], [[ROW2, nT], [1, 40]]),
                            in0=ap(g_t[:], [[ROW2, nT], [1, 40]]),
                            in1=ap(pe[:], [[1, nT], [0, 40]], extra_off=tlo),
                            op=Alu.mult)
                        nc.vector.tensor_copy(out=g_t[:, :, 40:41],
                                              in_=pev[:, tlo:tlo + nT, :])
                    psb = l2ps.tile([128, 41], f32)
                    for t in range(T):
                        g_t, tt = (glo, t) if t < T_LO else (ghi, t - T_LO)
                        nc.tensor.matmul(out=psb[:],
                                         lhsT=sk[:, t * 128:(t + 1) * 128],
                                         rhs=g_t[:, tt, 0:41],
                                         start=(t == 0), stop=(t == T - 1))
                    dn = postp2.tile([128, 1], f32, tag="dn2")
                    nc.vector.tensor_scalar_add(out=dn[:], in0=psb[:, 40:41], scalar1=1e-16)
                    rcp = postp2.tile([128, 1], f32, tag="rcp2")
                    nc.vector.reciprocal(out=rcp[:], in_=dn[:])
                    o2 = postp2.tile([128, OUT_DIM], f32, tag="o2")
                    nc.vector.tensor_scalar(out=o2[:], in0=psb[:, 0:40], scalar1=rcp[:, 0:1],
                                            scalar2=None, op0=Alu.mult)
                    nc.vector.tensor_tensor(out=o2[:], in0=o2[:], in1=b2r_t[:], op=Alu.add)
                    mx = postp2.tile([128, 1], f32, tag="mx")
                    nc.vector.tensor_reduce(out=mx[:], in_=o2[:], op=Alu.max,
                                            axis=mybir.AxisListType.X)
                    mxn = postp2.tile([128, 1], f32, tag="mxn")
                    nc.vector.tensor_scalar_mul(out=mxn[:], in0=mx[:], scalar1=-1.0)
                    ex = postp2.tile([128, OUT_DIM], f32, tag="ex")
                    nc.scalar.activation(out=ex[:], in_=o2[:], func=Act.Exp, bias=mxn[:, 0:1])
                    sm = postp2.tile([128, 1], f32, tag="sm")
                    nc.vector.tensor_reduce(out=sm[:], in_=ex[:], op=Alu.add,
                                            axis=mybir.AxisListType.X)
                    lns = postp2.tile([128, 1], f32, tag="lns")
                    nc.scalar.activation(out=lns[:], in_=sm[:], func=Act.Ln)
                    tot = postp2.tile([128, 1], f32, tag="tot")
                    nc.vector.tensor_tensor(out=tot[:], in0=mx[:], in1=lns[:], op=Alu.add)
                    o2f = postp2.tile([128, OUT_DIM], f32, tag="o2f")
                    nc.vector.tensor_scalar(out=o2f[:], in0=o2[:], scalar1=tot[:, 0:1],
                                            scalar2=None, op0=Alu.subtract)
                    nc.sync.dma_start(out=out_d[b * 128:(b + 1) * 128, :], in_=o2f[:])
    return nc


_CACHE = {}


LAST_EXEC_NS = -1


def kernel(**inputs):
    return _run(inputs, "full")


def _run(inputs, phases, trace=False, tmpdir=None):
    from concourse.bass_utils import run_bass_kernel_spmd
    shared, percore, (T_LO, T_HI), pos = _prep(inputs)
    key = (T_LO, T_HI, phases)
    if key not in _CACHE:
        nc = _build(T_LO, T_HI, phases)
        nc.compile()
        _CACHE[key] = nc
    nc = _CACHE[key]
    in_maps = []
    for c in range(NC):
        m = dict(shared)
        m.update(percore[c])
        in_maps.append(m)
    res = run_bass_kernel_spmd(nc, in_maps, list(range(NC)), trace=trace, tmpdir=tmpdir)
    global LAST_EXEC_NS
    if res.exec_time_ns is not None:
        LAST_EXEC_NS = res.exec_time_ns
    full = np.concatenate([res.results[c]["out"] for c in range(NC)], axis=0)
    return np.ascontiguousarray(full[pos]).astype(np.float32)


# revision 15
# speedup vs baseline: 2.2414x; 1.2596x over previous
"""Two-layer GAT on 8 trn2 NeuronCores (SPMD Bass kernel), v2.

Profiling on trn2 showed the bottleneck is gpsimd descriptor generation for
dma_gather (~8 ns per gathered row, serialized on the Pool engine), with the
per-tile DVE one-hot builds (~900 ns each) second.  v2 therefore minimizes
gathered ROW COUNT and ships the one-hot matrices from the host:

- Nodes are permuted into 392 degree-balanced bins of 128 (8 cores x 49
  blocks); edges are assigned to the destination's block and split into two
  streams by source half (int16 gather indices).  A 2-D greedy pass balances
  per-bin (lo, hi) stream loads to minimize tile count T.
- Per (block, tile): scatter one-hot S[p,q]=(slot(p)==q) and its transpose
  S_T are shipped as fp8 host data (exact 0/1; fp8 lhsT x bf16 rhs matmul
  verified exact on hw).  Pad slots gather row 0 and carry zero one-hot
  columns, so they contribute nothing.
- dst-side attention coefficients are never gathered per edge: ad1 per edge
  comes from T matmuls (lhsT=S_T, rhs=block's own ad1 [128,4]); ad2 per edge
  is computed the same way in L1 (S_T still in SBUF) and stashed in SBUF for
  L2.  The block's own ad1 is fetched with a single 128-row dma_gather from
  a 2-nodes-per-row table + parity select (SPMD-uniform addressing).
- Phase A (x @ [W1 | W1 a_s | W1 a_d]) runs in bf16 (fp32r measured 534
  ns/matmul; bf16 ~110), x shipped bf16 (halves DMA).
- Layer-2 table is one bf16 AllGather output [NTOT, 128] (256B rows); lo/hi
  gathers address row-range views; no local rebuild pass.
"""
import numpy as np
import ml_dtypes

N = 50000
IN_DIM = 256
HID = 64
HEADS = 4
OUT_DIM = 40
E = 800000
NEG = 0.2

NC = 8
BLOCKS_PER_CORE = 49
NBLK = NC * BLOCKS_PER_CORE            # 392
NODES_PER_CORE = BLOCKS_PER_CORE * 128  # 6272
NTOT = NBLK * 128                       # 50176
HALF = NTOT // 2                        # 25088
ROW1 = 192    # f32 words per L1 table row (768B): h bf16[256] | as1 f32[4] | pad
ROW2 = 128    # bf16 words per L2 table row (256B): h2 bf16[40] | as2 bf16 | pad


def _prep(inputs):
    x = np.asarray(inputs["x"], dtype=np.float32)
    ei = np.asarray(inputs["edge_index"])
    W1 = np.asarray(inputs["W1"], dtype=np.float32)
    as1 = np.asarray(inputs["att_src1"], dtype=np.float32)
    ad1 = np.asarray(inputs["att_dst1"], dtype=np.float32)
    b1 = np.asarray(inputs["b1"], dtype=np.float32)
    W2 = np.asarray(inputs["W2"], dtype=np.float32)
    as2 = np.asarray(inputs["att_src2"], dtype=np.float32)
    ad2 = np.asarray(inputs["att_dst2"], dtype=np.float32)
    b2 = np.asarray(inputs["b2"], dtype=np.float32)

    src = np.concatenate([ei[0], np.arange(N, dtype=ei.dtype)]).astype(np.int64)
    dst = np.concatenate([ei[1], np.arange(N, dtype=ei.dtype)]).astype(np.int64)

    # ---- pass 1: split nodes into lo/hi halves by degree snake (as v1) ----
    deg = np.bincount(dst, minlength=N)
    order = np.argsort(-deg, kind="stable")
    half_of = np.empty(N, dtype=np.int8)   # 0 = lo half, 1 = hi half
    nfull = N // NBLK
    bins0 = np.empty(N, dtype=np.int64)
    for r in range(nfull + 1):
        lo = r * NBLK
        hi = min(lo + NBLK, N)
        if lo >= hi:
            break
        nodes = order[lo:hi]
        b = np.arange(hi - lo)
        if r % 2 == 1:
            b = NBLK - 1 - b
        bins0[nodes] = b
    half_of[:] = (bins0 >= NBLK // 2)

    # ---- pass 2: per-node (lo, hi) in-degree, greedy 2-D balance ----
    src_half = half_of[src]
    dlo = np.bincount(dst[src_half == 0], minlength=N)
    dhi = np.bincount(dst[src_half == 1], minlength=N)
    pos = np.empty(N, dtype=np.int64)
    HB = NBLK // 2
    for h in (0, 1):
        nodes = np.where(half_of == h)[0]
        nodes = nodes[np.argsort(-(dlo[nodes] + dhi[nodes]), kind="stable")]
        load_lo = np.zeros(HB, dtype=np.int64)
        load_hi = np.zeros(HB, dtype=np.int64)
        count = np.zeros(HB, dtype=np.int64)
        slot_base = (np.arange(HB) + h * HB) * 128
        for n in nodes:
            score = np.maximum(load_lo + dlo[n], load_hi + dhi[n])
            score[count >= 128] = 1 << 60
            g = int(np.argmin(score))
            pos[n] = slot_base[g] + count[g]
            count[g] += 1
            load_lo[g] += dlo[n]
            load_hi[g] += dhi[n]

    spos = pos[src]
    dpos = pos[dst]
    gbin = dpos // 128
    slot = dpos % 128
    is_lo = spos < HALF
    srow = np.where(is_lo, spos, spos - HALF)

    cnt_lo = np.bincount(gbin[is_lo], minlength=NBLK)
    cnt_hi = np.bincount(gbin[~is_lo], minlength=NBLK)
    T_LO = int(np.ceil(cnt_lo.max() / 128))
    T_HI = int(np.ceil(cnt_hi.max() / 128))
    T = T_LO + T_HI

    def build_canvas(mask, ntiles):
        n_pad = ntiles * 128
        c_src = np.zeros((NBLK, n_pad), dtype=np.int64)     # pad -> row 0
        c_slot = np.full((NBLK, n_pad), -1, dtype=np.int64)  # pad -> -1
        g = gbin[mask]
        o = np.argsort(g, kind="stable")
        g = g[o]
        starts = np.zeros(NBLK + 1, dtype=np.int64)
        np.cumsum(np.bincount(g, minlength=NBLK), out=starts[1:])
        within = np.arange(g.shape[0]) - starts[g]
        flat = g * n_pad + within
        c_src.reshape(-1)[flat] = srow[mask][o]
        c_slot.reshape(-1)[flat] = slot[mask][o]
        return c_src, c_slot

    clo_src, clo_slot = build_canvas(is_lo, T_LO)
    chi_src, chi_slot = build_canvas(~is_lo, T_HI)
    c_slot = np.concatenate([clo_slot.reshape(NBLK, T_LO, 128),
                             chi_slot.reshape(NBLK, T_HI, 128)], axis=1)

    def wrap_idx(canvas, ntiles):
        n = ntiles * 128
        w = canvas.reshape(NBLK, n // 16, 16).transpose(0, 2, 1).astype(np.int16)
        return np.tile(w, (1, 8, 1)).copy()  # [NBLK, 128, n/16]

    idx_lo = wrap_idx(clo_src, T_LO)
    idx_hi = wrap_idx(chi_src, T_HI)

    # block-ad gather indices: 128 idx = (gbin*128 + p) >> 1, wrapped
    padr = (np.arange(NBLK)[:, None] * 128 + np.arange(128)[None, :]) >> 1
    idx_ad = wrap_idx(padr.reshape(NBLK, 128), 1)  # [NBLK, 128, 8]
    idx_all = np.concatenate([idx_lo, idx_hi, idx_ad], axis=2)  # [NBLK,128,T*8+8]

    # one-hot stacks, fp8 (exact 0/1). pad slots (c_slot=-1) stay all-zero.
    valid = c_slot >= 0                        # [NBLK, T, 128]
    g_i, t_i, p_i = np.nonzero(valid)
    q_i = c_slot[valid]
    stks = np.zeros((NBLK, 128, 2 * T * 128), dtype=ml_dtypes.float8_e4m3fn)
    one = np.float32(1.0).astype(ml_dtypes.float8_e4m3fn)
    stks[g_i, p_i, t_i * 128 + q_i] = one                 # S
    stks[g_i, q_i, T * 128 + t_i * 128 + p_i] = one       # S_T

    # weights (bf16)
    W1e = np.zeros((IN_DIM, 264), dtype=np.float32)
    W1e[:, :256] = W1
    for h in range(HEADS):
        W1e[:, 256 + h] = W1[:, h * HID:(h + 1) * HID] @ as1[h]
        W1e[:, 260 + h] = W1[:, h * HID:(h + 1) * HID] @ ad1[h]
    W2e = np.zeros((IN_DIM, 42), dtype=np.float32)
    W2e[:, :40] = W2
    W2e[:, 40] = W2 @ as2[0]
    W2e[:, 41] = W2 @ ad2[0]

    xT = np.zeros((IN_DIM, NTOT), dtype=np.float32)
    xT[:, pos] = x.T

    b1r = np.tile(b1[None, :], (128, 1)).astype(np.float32).copy()
    b2r = np.tile(b2[None, :], (128, 1)).astype(np.float32).copy()
    ident = np.eye(128, dtype=np.float32)
    parity = (np.arange(128, dtype=np.float32) % 2).reshape(128, 1).copy()

    shared = dict(xTb=xT.astype(ml_dtypes.bfloat16),
                  W1e=W1e.astype(ml_dtypes.bfloat16),
                  W2e=W2e.astype(ml_dtypes.bfloat16),
                  b1r=b1r, b2r=b2r, ident=ident, parity=parity)
    percore = []
    for c in range(NC):
        s = slice(c * BLOCKS_PER_CORE, (c + 1) * BLOCKS_PER_CORE)
        percore.append(dict(idx_all=idx_all[s], stks=stks[s]))
    return shared, percore, (T_LO, T_HI), pos


def _build(T_LO, T_HI, phases="full"):
    import concourse.bass as bass
    import concourse.bacc as bacc
    import concourse.mybir as mybir
    import concourse.tile as tile

    f32 = mybir.dt.float32
    bf16 = mybir.dt.bfloat16
    fp8 = mybir.dt.float8e4
    i16 = mybir.dt.int16
    Alu = mybir.AluOpType
    Act = mybir.ActivationFunctionType
    T = T_LO + T_HI

    nc = bacc.Bacc("TRN2", target_bir_lowering=False, debug=False,
                   num_devices=NC, num_swdge_queues=4)

    xTb = nc.dram_tensor("xTb", [IN_DIM, NTOT], bf16, kind="ExternalInput")
    W1e_d = nc.dram_tensor("W1e", [IN_DIM, 264], bf16, kind="ExternalInput")
    W2e_d = nc.dram_tensor("W2e", [IN_DIM, 42], bf16, kind="ExternalInput")
    b1r_d = nc.dram_tensor("b1r", [128, 256], f32, kind="ExternalInput")
    b2r_d = nc.dram_tensor("b2r", [128, OUT_DIM], f32, kind="ExternalInput")
    ident_d = nc.dram_tensor("ident", [128, 128], f32, kind="ExternalInput")
    par_d = nc.dram_tensor("parity", [128, 1], f32, kind="ExternalInput")
    idx_all_d = nc.dram_tensor("idx_all", [BLOCKS_PER_CORE, 128, T * 8 + 8], i16, kind="ExternalInput")
    stks_d = nc.dram_tensor("stks", [BLOCKS_PER_CORE, 128, 2 * T * 128], fp8, kind="ExternalInput")
    out_d = nc.dram_tensor("out", [NODES_PER_CORE, OUT_DIM], f32, kind="ExternalOutput")

    def ap(view, dims, extra_off=0):
        return bass.AP(view.tensor, view.offset + extra_off, [list(view.ap[0])] + dims)

    with tile.TileContext(nc) as tc:
        with tc.tile_pool(name="dram", bufs=1, space="DRAM") as dram, \
             tc.tile_pool(name="const", bufs=1) as cpool, \
             tc.tile_pool(name="stash", bufs=1) as stash:
            tabL1_lo = dram.tile([HALF, ROW1], f32)
            tabL1_hi = dram.tile([HALF, ROW1], f32)
            blockad = dram.tile([NTOT // 2, 64], f32)
            h2shard = dram.tile([NODES_PER_CORE, ROW2], bf16)
            tabL2 = dram.tile([NTOT, ROW2], bf16, addr_space="Shared")

            w1e0 = cpool.tile([128, 264], bf16)
            w1e1 = cpool.tile([128, 264], bf16)
            nc.sync.dma_start(out=w1e0[:], in_=W1e_d[0:128, :])
            nc.sync.dma_start(out=w1e1[:], in_=W1e_d[128:256, :])
            w2e0 = cpool.tile([128, 42], bf16)
            w2e1 = cpool.tile([128, 42], bf16)
            nc.sync.dma_start(out=w2e0[:], in_=W2e_d[0:128, :])
            nc.sync.dma_start(out=w2e1[:], in_=W2e_d[128:256, :])
            b1r_t = cpool.tile([128, 256], f32)
            b2r_t = cpool.tile([128, OUT_DIM], f32)
            nc.sync.dma_start(out=b1r_t[:], in_=b1r_d[:])
            nc.sync.dma_start(out=b2r_t[:], in_=b2r_d[:])
            id_t = cpool.tile([128, 128], f32)
            nc.sync.dma_start(out=id_t[:], in_=ident_d[:])
            par_t = cpool.tile([128, 1], f32)
            nc.sync.dma_start(out=par_t[:], in_=par_d[:])
            ad2st = stash.tile([128, BLOCKS_PER_CORE * T], f32)

            # ---------------- Phase A (4 blocks per iteration) ----------------
            with tc.tile_pool(name="pa_x", bufs=3) as pax, \
                 tc.tile_pool(name="pa_ps", bufs=2, space="PSUM") as paps, \
                 tc.tile_pool(name="pa_row", bufs=3) as parow, \
                 tc.tile_pool(name="pa_ad", bufs=3) as paad:
                for n4 in range(NBLK // 4):
                    # one DMA: x for 4 blocks, both K halves -> [128, 4, 2, 128]
                    xt = pax.tile([128, 2, 4, 128], bf16, tag="xt")
                    xv = xTb[0:128, 0:128]
                    for k in range(2):
                        nc.sync.dma_start(
                            out=xt[:, k, :, :],
                            in_=bass.AP(xv.tensor, xv.offset + k * 128 * NTOT + n4 * 512,
                                        [[NTOT, 128], [128, 4], [1, 128]]))
                    row = parow.tile([128, 4, 130], f32, tag="row")
                    adsb = paad.tile([128, 4, 4], f32, tag="adsb")
                    for j in range(4):
                        ps = paps.tile([128, 264], f32, tag=f"ps{j}")
                        nc.tensor.matmul(out=ps[:], lhsT=xt[:, 0, j, :], rhs=w1e0[:],
                                         start=True, stop=False)
                        nc.tensor.matmul(out=ps[:], lhsT=xt[:, 1, j, :], rhs=w1e1[:],
                                         start=False, stop=True)
                        nc.vector.tensor_copy(out=row[:, j, 0:130].bitcast(bf16),
                                              in_=ps[:, 0:260])
                        nc.vector.tensor_copy(out=adsb[:, j, :], in_=ps[:, 260:264])
                    tab = tabL1_lo if n4 < NBLK // 8 else tabL1_hi
                    r0 = (n4 * 512) % HALF
                    tv = tab[:]
                    nc.sync.dma_start(
                        out=bass.AP(tv.tensor, tv.offset + r0 * ROW1,
                                    [[ROW1, 128], [ROW1 * 128, 4], [1, 130]]),
                        in_=row[:])
                    bv = blockad[:]
                    for j in range(4):
                        nc.sync.dma_start(
                            out=bass.AP(bv.tensor, bv.offset + (n4 * 4 + j) * 64 * 64,
                                        [[64, 64], [4, 2], [1, 4]]),
                            in_=adsb[:, j, :])

            if phases == "A":
                return nc

            # ---------------- L1 edge phase (+ fused layer-2 projection) ----
            l1n = BLOCKS_PER_CORE
            if phases.startswith("L1:"):
                l1n = int(phases.split(":")[1])
            with tc.tile_pool(name="g1", bufs=3) as g1p, \
                 tc.tile_pool(name="gidx", bufs=3) as gip, \
                 tc.tile_pool(name="sstk", bufs=3) as ssp, \
                 tc.tile_pool(name="gad", bufs=3) as gadp, \
                 tc.tile_pool(name="scr", bufs=3) as scrp, \
                 tc.tile_pool(name="post", bufs=3) as postp, \
                 tc.tile_pool(name="l1ps", bufs=2, space="PSUM") as l1ps, \
                 tc.tile_pool(name="tps", bufs=2, space="PSUM") as tps, \
                 tc.tile_pool(name="a2ps", bufs=2, space="PSUM") as a2ps, \
                 tc.tile_pool(name="adps", bufs=2, space="PSUM") as adpsp:
                for b in range(l1n):
                    ix = gip.tile([128, T * 8 + 8], i16, tag="ix")
                    nc.sync.dma_start(out=ix[:], in_=idx_all_d[b])
                    il = ix[:, 0:T_LO * 8]
                    ih = ix[:, T_LO * 8:T * 8]
                    ia = ix[:, T * 8:T * 8 + 8]
                    sks = ssp.tile([128, 2 * T * 128], fp8, tag="sks")
                    nc.sync.dma_start(out=sks[:], in_=stks_d[b])
                    sk = sks[:, 0:T * 128]
                    stk = sks[:, T * 128:2 * T * 128]

                    # block-ad gather first: the ad-matmul chain overlaps row gathers
                    gad = gadp.tile([128, 1, 64], f32, tag="gad")
                    nc.gpsimd.dma_gather(
                        out_ap=gad[:], in_ap=blockad[:], idxs_ap=ia,
                        num_idxs=128, num_idxs_reg=128, elem_size=64,
                        queue_num=3)
                    glo = g1p.tile([128, T_LO, ROW1], f32, tag="glo")
                    ghi = g1p.tile([128, T_HI, ROW1], f32, tag="ghi")
                    qn = 0
                    for g_t, tab, idxs, nt_s in ((glo, tabL1_lo, il, T_LO),
                                                 (ghi, tabL1_hi, ih, T_HI)):
                        for c0 in range(0, nt_s, 8):
                            cn = min(8, nt_s - c0)
                            nc.gpsimd.dma_gather(
                                out_ap=g_t[:, c0:c0 + cn, :], in_ap=tab[:],
                                idxs_ap=idxs[:, c0 * 8:(c0 + cn) * 8],
                                num_idxs=cn * 128, num_idxs_reg=cn * 128,
                                elem_size=ROW1, queue_num=qn % 3)
                            qn += 1

                    # block ad1 via parity select: ad = adA + par*(adB - adA)
                    dfa = scrp.tile([128, 4], f32, tag="dfa")
                    nc.vector.tensor_tensor(out=dfa[:], in0=gad[:, 0, 4:8],
                                            in1=gad[:, 0, 0:4], op=Alu.subtract)
                    nc.vector.tensor_tensor(
                        out=dfa[:], in0=dfa[:],
                        in1=ap(par_t[:], [[0, 4]]), op=Alu.mult)
                    adblk = scrp.tile([128, 4], bf16, tag="adblk")
                    nc.vector.tensor_tensor(out=adblk[:], in0=gad[:, 0, 0:4],
                                            in1=dfa[:], op=Alu.add)

                    # per-edge ad1: T matmuls lhsT=S_T fp8
                    adp = adpsp.tile([128, T * 4 + T], f32)
                    for t in range(T):
                        nc.tensor.matmul(out=adp[:, t * 4:(t + 1) * 4],
                                         lhsT=stk[:, t * 128:(t + 1) * 128],
                                         rhs=adblk[:], start=True, stop=True)

                    # logits = as + ad, lrelu, exp
                    pe = scrp.tile([128, T * 4], f32, tag="pe")
                    pev = pe[:].rearrange("p (t f) -> p t f", f=4)
                    adv_ = adp[:, 0:T * 4].rearrange("p (t f) -> p t f", f=4)
                    nc.vector.tensor_tensor(
                        out=pev[:, 0:T_LO, :], in0=adv_[:, 0:T_LO, :],
                        in1=ap(glo[:].bitcast(bf16), [[384, T_LO], [1, 4]], extra_off=256),
                        op=Alu.add)
                    nc.vector.tensor_tensor(
                        out=pev[:, T_LO:T, :], in0=adv_[:, T_LO:T, :],
                        in1=ap(ghi[:].bitcast(bf16), [[384, T_HI], [1, 4]], extra_off=256),
                        op=Alu.add)
                    u = scrp.tile([128, T * 4], f32, tag="u")
                    nc.vector.tensor_scalar_mul(out=u[:], in0=pe[:], scalar1=NEG)
                    nc.vector.tensor_tensor(out=pe[:], in0=pe[:], in1=u[:], op=Alu.max)
                    nc.scalar.activation(out=pe[:], in_=pe[:], func=Act.Exp)
                    # p -> bf16 into rows at word 128
                    nc.vector.tensor_copy(out=glo[:, :, 128:130].bitcast(bf16),
                                          in_=pev[:, 0:T_LO, :])
                    nc.vector.tensor_copy(out=ghi[:, :, 128:130].bitcast(bf16),
                                          in_=pev[:, T_LO:T, :])
                    # h *= p (per head), bf16
                    for g_t, nT in ((glo, T_LO), (ghi, T_HI)):
                        hb = g_t[:].bitcast(bf16)
                        nc.vector.tensor_tensor(
                            out=ap(hb, [[384, nT], [64, 4], [1, 64]]),
                            in0=ap(hb, [[384, nT], [64, 4], [1, 64]]),
                            in1=ap(hb, [[384, nT], [1, 4], [0, 64]], extra_off=256),
                            op=Alu.mult)

                    # scatter-add via one-hot matmuls
                    psb = l1ps.tile([128, 260], f32)
                    for t in range(T):
                        g_t, tt = (glo, t) if t < T_LO else (ghi, t - T_LO)
                        nc.tensor.matmul(out=psb[:],
                                         lhsT=sk[:, t * 128:(t + 1) * 128],
                                         rhs=g_t[:, tt, 0:130].bitcast(bf16),
                                         start=(t == 0), stop=(t == T - 1))
                    # divide + bias + ELU
                    dn = postp.tile([128, 4], f32, tag="dn")
                    nc.vector.tensor_scalar_add(out=dn[:], in0=psb[:, 256:260], scalar1=1e-16)
                    rcp = postp.tile([128, 4], f32, tag="rcp")
                    nc.vector.reciprocal(out=rcp[:], in_=dn[:])
                    o1 = postp.tile([128, 256], f32, tag="o1")
                    o1v = o1[:].rearrange("p (h c) -> p h c", h=4)
                    nc.vector.tensor_tensor(out=o1v, in0=psb[:, 0:256].rearrange("p (h c) -> p h c", h=4),
                                            in1=ap(rcp[:], [[1, 4], [0, 64]]), op=Alu.mult)
                    nc.vector.tensor_tensor(out=o1[:], in0=o1[:], in1=b1r_t[:], op=Alu.add)
                    em = postp.tile([128, 256], f32, tag="em")
                    nc.vector.tensor_scalar_min(out=em[:], in0=o1[:], scalar1=0.0)
                    nc.scalar.activation(out=em[:], in_=em[:], func=Act.Exp)
                    nc.vector.tensor_scalar_max(out=o1[:], in0=o1[:], scalar1=0.0)
                    nc.vector.tensor_tensor(out=o1[:], in0=o1[:], in1=em[:], op=Alu.add)
                    nc.vector.tensor_scalar_add(out=o1[:], in0=o1[:], scalar1=-1.0)
                    # layer-2 projection: h2 = o1 @ W2e (bf16)
                    ps2 = a2ps.tile([128, 42], f32)
                    for c_i, w2c in ((0, w2e0), (1, w2e1)):
                        pst = tps.tile([128, 128], f32)
                        nc.tensor.transpose(out=pst[:], in_=o1[:, c_i * 128:(c_i + 1) * 128],
                                            identity=id_t[:])
                        tsb = postp.tile([128, 128], bf16, tag=f"tsb{c_i}")
                        nc.vector.tensor_copy(out=tsb[:], in_=pst[:])
                        nc.tensor.matmul(out=ps2[:], lhsT=tsb[:], rhs=w2c[:],
                                         start=(c_i == 0), stop=(c_i == 1))
                    h2row = postp.tile([128, 41], bf16, tag="h2row")
                    nc.vector.tensor_copy(out=h2row[:], in_=ps2[:, 0:41])
                    nc.sync.dma_start(out=h2shard[b * 128:(b + 1) * 128, 0:41], in_=h2row[:])
                    # per-edge ad2 via S_T (still in SBUF), stash for L2
                    adblk2 = postp.tile([128, 1], bf16, tag="adblk2")
                    nc.vector.tensor_copy(out=adblk2[:], in_=ps2[:, 41:42])
                    for t in range(T):
                        nc.tensor.matmul(out=adp[:, T * 4 + t:T * 4 + t + 1],
                                         lhsT=stk[:, t * 128:(t + 1) * 128],
                                         rhs=adblk2[:], start=True, stop=True)
                    nc.vector.tensor_copy(out=ad2st[:, b * T:(b + 1) * T],
                                          in_=adp[:, T * 4:T * 4 + T])

            if phases == "A1" or phases.startswith("L1:"):
                return nc

            # ---------------- AllGather ----------------
            nc.gpsimd.collective_compute(
                "AllGather", mybir.AluOpType.bypass,
                replica_groups=[list(range(NC))],
                ins=[h2shard[:]], outs=[tabL2[:]])

            if phases == "A1C":
                return nc

            # ---------------- L2 edge phase ----------------
            with tc.tile_pool(name="g2", bufs=3) as g2p, \
                 tc.tile_pool(name="gidx2", bufs=3) as gip2, \
                 tc.tile_pool(name="sstk2", bufs=3) as ssp2, \
                 tc.tile_pool(name="scr2", bufs=3) as scrp2, \
                 tc.tile_pool(name="post2", bufs=3) as postp2, \
                 tc.tile_pool(name="l2ps", bufs=2, space="PSUM") as l2ps:
                for b in range(BLOCKS_PER_CORE):
                    ix = gip2.tile([128, T * 8], i16, tag="ix2")
                    nc.sync.dma_start(out=ix[:], in_=idx_all_d[b][:, 0:T * 8])
                    il = ix[:, 0:T_LO * 8]
                    ih = ix[:, T_LO * 8:T * 8]
                    sk = ssp2.tile([128, T * 128], fp8, tag="sk2")
                    nc.sync.dma_start(out=sk[:], in_=stks_d[b][:, 0:T * 128])

                    glo = g2p.tile([128, T_LO, ROW2], bf16, tag="glo2")
                    ghi = g2p.tile([128, T_HI, ROW2], bf16, tag="ghi2")
                    qn = 0
                    for g_t, r0, r1, idxs, nt_s in (
                            (glo, 0, HALF, il, T_LO),
                            (ghi, HALF, NTOT, ih, T_HI)):
                        for c0 in range(0, nt_s, 8):
                            cn = min(8, nt_s - c0)
                            nc.gpsimd.dma_gather(
                                out_ap=g_t[:, c0:c0 + cn, :],
                                in_ap=tabL2[r0:r1, :],
                                idxs_ap=idxs[:, c0 * 8:(c0 + cn) * 8],
                                num_idxs=cn * 128, num_idxs_reg=cn * 128,
                                elem_size=ROW2, queue_num=qn % 4)
                            qn += 1

                    pe = scrp2.tile([128, T], f32, tag="pe2")
                    pev = pe[:].rearrange("p (t f) -> p t f", f=1)
                    adv_ = ad2st[:, b * T:(b + 1) * T].rearrange("p (t f) -> p t f", f=1)
                    nc.vector.tensor_tensor(out=pev[:, 0:T_LO, :], in0=adv_[:, 0:T_LO, :],
                                            in1=glo[:, :, 40:41], op=Alu.add)
                    nc.vector.tensor_tensor(out=pev[:, T_LO:T, :], in0=adv_[:, T_LO:T, :],
                                            in1=ghi[:, :, 40:41], op=Alu.add)
                    u = scrp2.tile([128, T], f32, tag="u2")
                    nc.vector.tensor_scalar_mul(out=u[:], in0=pe[:], scalar1=NEG)
                    nc.vector.tensor_tensor(out=pe[:], in0=pe[:], in1=u[:], op=Alu.max)
                    nc.scalar.activation(out=pe[:], in_=pe[:], func=Act.Exp)
                    # h2 *= p2 ; write p2 into word 40
                    for g_t, tlo, nT in ((glo, 0, T_LO), (ghi, T_LO, T_HI)):
                        nc.vector.tensor_tensor(
                            out=ap(g_t[:], [[ROW2, nT], [1, 40]]),
                            in0=ap(g_t[:], [[ROW2, nT], [1, 40]]),
                            in1=ap(pe[:], [[1, nT], [0, 40]], extra_off=tlo),
                            op=Alu.mult)
                        nc.vector.tensor_copy(out=g_t[:, :, 40:41],
                                              in_=pev[:, tlo:tlo + nT, :])
                    psb = l2ps.tile([128, 41], f32)
                    for t in range(T):
                        g_t, tt = (glo, t) if t < T_LO else (ghi, t - T_LO)
                        nc.tensor.matmul(out=psb[:],
                                         lhsT=sk[:, t * 128:(t + 1) * 128],
                                         rhs=g_t[:, tt, 0:41],
                                         start=(t == 0), stop=(t == T - 1))
                    dn = postp2.tile([128, 1], f32, tag="dn2")
                    nc.vector.tensor_scalar_add(out=dn[:], in0=psb[:, 40:41], scalar1=1e-16)
                    rcp = postp2.tile([128, 1], f32, tag="rcp2")
                    nc.vector.reciprocal(out=rcp[:], in_=dn[:])
                    o2 = postp2.tile([128, OUT_DIM], f32, tag="o2")
                    nc.vector.tensor_scalar(out=o2[:], in0=psb[:, 0:40], scalar1=rcp[:, 0:1],
                                            scalar2=None, op0=Alu.mult)
                    nc.vector.tensor_tensor(out=o2[:], in0=o2[:], in1=b2r_t[:], op=Alu.add)
                    mx = postp2.tile([128, 1], f32, tag="mx")
                    nc.vector.tensor_reduce(out=mx[:], in_=o2[:], op=Alu.max,
                                            axis=mybir.AxisListType.X)
                    mxn = postp2.tile([128, 1], f32, tag="mxn")
                    nc.vector.tensor_scalar_mul(out=mxn[:], in0=mx[:], scalar1=-1.0)
                    ex = postp2.tile([128, OUT_DIM], f32, tag="ex")
                    nc.scalar.activation(out=ex[:], in_=o2[:], func=Act.Exp, bias=mxn[:, 0:1])
                    sm = postp2.tile([128, 1], f32, tag="sm")
                    nc.vector.tensor_reduce(out=sm[:], in_=ex[:], op=Alu.add,
                                            axis=mybir.AxisListType.X)
                    lns = postp2.tile([128, 1], f32, tag="lns")
                    nc.scalar.activation(out=lns[:], in_=sm[:], func=Act.Ln)
                    tot = postp2.tile([128, 1], f32, tag="tot")
                    nc.vector.tensor_tensor(out=tot[:], in0=mx[:], in1=lns[:], op=Alu.add)
                    o2f = postp2.tile([128, OUT_DIM], f32, tag="o2f")
                    nc.vector.tensor_scalar(out=o2f[:], in0=o2[:], scalar1=tot[:, 0:1],
                                            scalar2=None, op0=Alu.subtract)
                    nc.sync.dma_start(out=out_d[b * 128:(b + 1) * 128, :], in_=o2f[:])
    return nc


_CACHE = {}


LAST_EXEC_NS = -1


def kernel(**inputs):
    return _run(inputs, "full")


def _run(inputs, phases, trace=False, tmpdir=None):
    from concourse.bass_utils import run_bass_kernel_spmd
    shared, percore, (T_LO, T_HI), pos = _prep(inputs)
    key = (T_LO, T_HI, phases)
    if key not in _CACHE:
        nc = _build(T_LO, T_HI, phases)
        nc.compile()
        _CACHE[key] = nc
    nc = _CACHE[key]
    in_maps = []
    for c in range(NC):
        m = dict(shared)
        m.update(percore[c])
        in_maps.append(m)
    res = run_bass_kernel_spmd(nc, in_maps, list(range(NC)), trace=trace, tmpdir=tmpdir)
    global LAST_EXEC_NS
    if res.exec_time_ns is not None:
        LAST_EXEC_NS = res.exec_time_ns
    full = np.concatenate([res.results[c]["out"] for c in range(NC)], axis=0)
    return np.ascontiguousarray(full[pos]).astype(np.float32)


# revision 16
# speedup vs baseline: 2.2791x; 1.0168x over previous
"""Two-layer GAT on 8 trn2 NeuronCores (SPMD Bass kernel), v2.

Profiling on trn2 showed the bottleneck is gpsimd descriptor generation for
dma_gather (~8 ns per gathered row, serialized on the Pool engine), with the
per-tile DVE one-hot builds (~900 ns each) second.  v2 therefore minimizes
gathered ROW COUNT and ships the one-hot matrices from the host:

- Nodes are permuted into 392 degree-balanced bins of 128 (8 cores x 49
  blocks); edges are assigned to the destination's block and split into two
  streams by source half (int16 gather indices).  A 2-D greedy pass balances
  per-bin (lo, hi) stream loads to minimize tile count T.
- Per (block, tile): scatter one-hot S[p,q]=(slot(p)==q) and its transpose
  S_T are shipped as fp8 host data (exact 0/1; fp8 lhsT x bf16 rhs matmul
  verified exact on hw).  Pad slots gather row 0 and carry zero one-hot
  columns, so they contribute nothing.
- dst-side attention coefficients are never gathered per edge: ad1 per edge
  comes from T matmuls (lhsT=S_T, rhs=block's own ad1 [128,4]); ad2 per edge
  is computed the same way in L1 (S_T still in SBUF) and stashed in SBUF for
  L2.  The block's own ad1 is fetched with a single 128-row dma_gather from
  a 2-nodes-per-row table + parity select (SPMD-uniform addressing).
- Phase A (x @ [W1 | W1 a_s | W1 a_d]) runs in bf16 (fp32r measured 534
  ns/matmul; bf16 ~110), x shipped bf16 (halves DMA).
- Layer-2 table is one bf16 AllGather output [NTOT, 128] (256B rows); lo/hi
  gathers address row-range views; no local rebuild pass.
"""
import numpy as np
import ml_dtypes

N = 50000
IN_DIM = 256
HID = 64
HEADS = 4
OUT_DIM = 40
E = 800000
NEG = 0.2

NC = 8
BLOCKS_PER_CORE = 49
NBLK = NC * BLOCKS_PER_CORE            # 392
NODES_PER_CORE = BLOCKS_PER_CORE * 128  # 6272
NTOT = NBLK * 128                       # 50176
HALF = NTOT // 2                        # 25088
ROW1 = 192    # f32 words per L1 table row (768B): h bf16[256] | as1 f32[4] | pad
ROW2 = 128    # bf16 words per L2 table row (256B): h2 bf16[40] | as2 bf16 | pad


def _prep(inputs):
    x = np.asarray(inputs["x"], dtype=np.float32)
    ei = np.asarray(inputs["edge_index"])
    W1 = np.asarray(inputs["W1"], dtype=np.float32)
    as1 = np.asarray(inputs["att_src1"], dtype=np.float32)
    ad1 = np.asarray(inputs["att_dst1"], dtype=np.float32)
    b1 = np.asarray(inputs["b1"], dtype=np.float32)
    W2 = np.asarray(inputs["W2"], dtype=np.float32)
    as2 = np.asarray(inputs["att_src2"], dtype=np.float32)
    ad2 = np.asarray(inputs["att_dst2"], dtype=np.float32)
    b2 = np.asarray(inputs["b2"], dtype=np.float32)

    src = np.concatenate([ei[0], np.arange(N, dtype=ei.dtype)]).astype(np.int64)
    dst = np.concatenate([ei[1], np.arange(N, dtype=ei.dtype)]).astype(np.int64)

    # ---- pass 1: split nodes into lo/hi halves by degree snake (as v1) ----
    deg = np.bincount(dst, minlength=N)
    order = np.argsort(-deg, kind="stable")
    half_of = np.empty(N, dtype=np.int8)   # 0 = lo half, 1 = hi half
    nfull = N // NBLK
    bins0 = np.empty(N, dtype=np.int64)
    for r in range(nfull + 1):
        lo = r * NBLK
        hi = min(lo + NBLK, N)
        if lo >= hi:
            break
        nodes = order[lo:hi]
        b = np.arange(hi - lo)
        if r % 2 == 1:
            b = NBLK - 1 - b
        bins0[nodes] = b
    half_of[:] = (bins0 >= NBLK // 2)

    # ---- pass 2: per-node (lo, hi) in-degree, greedy 2-D balance ----
    src_half = half_of[src]
    dlo = np.bincount(dst[src_half == 0], minlength=N)
    dhi = np.bincount(dst[src_half == 1], minlength=N)
    pos = np.empty(N, dtype=np.int64)
    HB = NBLK // 2
    for h in (0, 1):
        nodes = np.where(half_of == h)[0]
        nodes = nodes[np.argsort(-(dlo[nodes] + dhi[nodes]), kind="stable")]
        load_lo = np.zeros(HB, dtype=np.int64)
        load_hi = np.zeros(HB, dtype=np.int64)
        count = np.zeros(HB, dtype=np.int64)
        slot_base = (np.arange(HB) + h * HB) * 128
        for n in nodes:
            score = np.maximum(load_lo + dlo[n], load_hi + dhi[n])
            score[count >= 128] = 1 << 60
            g = int(np.argmin(score))
            pos[n] = slot_base[g] + count[g]
            count[g] += 1
            load_lo[g] += dlo[n]
            load_hi[g] += dhi[n]

    spos = pos[src]
    dpos = pos[dst]
    gbin = dpos // 128
    slot = dpos % 128
    is_lo = spos < HALF
    srow = np.where(is_lo, spos, spos - HALF)

    cnt_lo = np.bincount(gbin[is_lo], minlength=NBLK)
    cnt_hi = np.bincount(gbin[~is_lo], minlength=NBLK)
    T_LO = int(np.ceil(cnt_lo.max() / 128))
    T_HI = int(np.ceil(cnt_hi.max() / 128))
    T = T_LO + T_HI

    def build_canvas(mask, ntiles):
        n_pad = ntiles * 128
        c_src = np.zeros((NBLK, n_pad), dtype=np.int64)     # pad -> row 0
        c_slot = np.full((NBLK, n_pad), -1, dtype=np.int64)  # pad -> -1
        g = gbin[mask]
        o = np.argsort(g, kind="stable")
        g = g[o]
        starts = np.zeros(NBLK + 1, dtype=np.int64)
        np.cumsum(np.bincount(g, minlength=NBLK), out=starts[1:])
        within = np.arange(g.shape[0]) - starts[g]
        flat = g * n_pad + within
        c_src.reshape(-1)[flat] = srow[mask][o]
        c_slot.reshape(-1)[flat] = slot[mask][o]
        return c_src, c_slot

    clo_src, clo_slot = build_canvas(is_lo, T_LO)
    chi_src, chi_slot = build_canvas(~is_lo, T_HI)
    c_slot = np.concatenate([clo_slot.reshape(NBLK, T_LO, 128),
                             chi_slot.reshape(NBLK, T_HI, 128)], axis=1)

    def wrap_idx(canvas, ntiles):
        n = ntiles * 128
        w = canvas.reshape(NBLK, n // 16, 16).transpose(0, 2, 1).astype(np.int16)
        return np.tile(w, (1, 8, 1)).copy()  # [NBLK, 128, n/16]

    idx_lo = wrap_idx(clo_src, T_LO)
    idx_hi = wrap_idx(chi_src, T_HI)

    # block-ad gather indices: 128 idx = (gbin*128 + p) >> 1, wrapped
    padr = (np.arange(NBLK)[:, None] * 128 + np.arange(128)[None, :]) >> 1
    idx_ad = wrap_idx(padr.reshape(NBLK, 128), 1)  # [NBLK, 128, 8]
    idx_all = np.concatenate([idx_lo, idx_hi, idx_ad], axis=2)  # [NBLK,128,T*8+8]

    # one-hot stacks, fp8 (exact 0/1). pad slots (c_slot=-1) stay all-zero.
    valid = c_slot >= 0                        # [NBLK, T, 128]
    g_i, t_i, p_i = np.nonzero(valid)
    q_i = c_slot[valid]
    stks = np.zeros((NBLK, 128, 2 * T * 128), dtype=ml_dtypes.float8_e4m3fn)
    one = np.float32(1.0).astype(ml_dtypes.float8_e4m3fn)
    stks[g_i, p_i, t_i * 128 + q_i] = one                 # S
    stks[g_i, q_i, T * 128 + t_i * 128 + p_i] = one       # S_T

    # weights (bf16)
    W1e = np.zeros((IN_DIM, 264), dtype=np.float32)
    W1e[:, :256] = W1
    for h in range(HEADS):
        W1e[:, 256 + h] = W1[:, h * HID:(h + 1) * HID] @ as1[h]
        W1e[:, 260 + h] = W1[:, h * HID:(h + 1) * HID] @ ad1[h]
    W2e = np.zeros((IN_DIM, 42), dtype=np.float32)
    W2e[:, :40] = W2
    W2e[:, 40] = W2 @ as2[0]
    W2e[:, 41] = W2 @ ad2[0]

    xT = np.zeros((IN_DIM, NTOT), dtype=np.float32)
    xT[:, pos] = x.T

    b1r = np.tile(b1[None, :], (128, 1)).astype(np.float32).copy()
    b2r = np.tile(b2[None, :], (128, 1)).astype(np.float32).copy()
    ident = np.eye(128, dtype=np.float32)
    parity = (np.arange(128, dtype=np.float32) % 2).reshape(128, 1).copy()

    shared = dict(xTb=xT.astype(ml_dtypes.bfloat16),
                  W1e=W1e.astype(ml_dtypes.bfloat16),
                  W2e=W2e.astype(ml_dtypes.bfloat16),
                  b1r=b1r, b2r=b2r, ident=ident, parity=parity)
    percore = []
    for c in range(NC):
        s = slice(c * BLOCKS_PER_CORE, (c + 1) * BLOCKS_PER_CORE)
        percore.append(dict(idx_all=idx_all[s], stks=stks[s]))
    return shared, percore, (T_LO, T_HI), pos


def _build(T_LO, T_HI, phases="full"):
    import concourse.bass as bass
    import concourse.bacc as bacc
    import concourse.mybir as mybir
    import concourse.tile as tile

    f32 = mybir.dt.float32
    bf16 = mybir.dt.bfloat16
    fp8 = mybir.dt.float8e4
    i16 = mybir.dt.int16
    Alu = mybir.AluOpType
    Act = mybir.ActivationFunctionType
    T = T_LO + T_HI

    nc = bacc.Bacc("TRN2", target_bir_lowering=False, debug=False,
                   num_devices=NC, num_swdge_queues=4)

    xTb = nc.dram_tensor("xTb", [IN_DIM, NTOT], bf16, kind="ExternalInput")
    W1e_d = nc.dram_tensor("W1e", [IN_DIM, 264], bf16, kind="ExternalInput")
    W2e_d = nc.dram_tensor("W2e", [IN_DIM, 42], bf16, kind="ExternalInput")
    b1r_d = nc.dram_tensor("b1r", [128, 256], f32, kind="ExternalInput")
    b2r_d = nc.dram_tensor("b2r", [128, OUT_DIM], f32, kind="ExternalInput")
    ident_d = nc.dram_tensor("ident", [128, 128], f32, kind="ExternalInput")
    par_d = nc.dram_tensor("parity", [128, 1], f32, kind="ExternalInput")
    idx_all_d = nc.dram_tensor("idx_all", [BLOCKS_PER_CORE, 128, T * 8 + 8], i16, kind="ExternalInput")
    stks_d = nc.dram_tensor("stks", [BLOCKS_PER_CORE, 128, 2 * T * 128], fp8, kind="ExternalInput")
    out_d = nc.dram_tensor("out", [NODES_PER_CORE, OUT_DIM], f32, kind="ExternalOutput")

    def ap(view, dims, extra_off=0):
        return bass.AP(view.tensor, view.offset + extra_off, [list(view.ap[0])] + dims)

    with tile.TileContext(nc) as tc:
        with tc.tile_pool(name="dram", bufs=1, space="DRAM") as dram, \
             tc.tile_pool(name="const", bufs=1) as cpool, \
             tc.tile_pool(name="stash", bufs=1) as stash:
            tabL1_lo = dram.tile([HALF, ROW1], f32)
            tabL1_hi = dram.tile([HALF, ROW1], f32)
            blockad = dram.tile([NTOT // 2, 64], f32)
            h2shard = dram.tile([NODES_PER_CORE, ROW2], bf16)
            tabL2 = dram.tile([NTOT, ROW2], bf16, addr_space="Shared")

            w1e0 = cpool.tile([128, 264], bf16)
            w1e1 = cpool.tile([128, 264], bf16)
            nc.sync.dma_start(out=w1e0[:], in_=W1e_d[0:128, :])
            nc.sync.dma_start(out=w1e1[:], in_=W1e_d[128:256, :])
            w2e0 = cpool.tile([128, 42], bf16)
            w2e1 = cpool.tile([128, 42], bf16)
            nc.sync.dma_start(out=w2e0[:], in_=W2e_d[0:128, :])
            nc.sync.dma_start(out=w2e1[:], in_=W2e_d[128:256, :])
            b1r_t = cpool.tile([128, 256], f32)
            b2r_t = cpool.tile([128, OUT_DIM], f32)
            nc.sync.dma_start(out=b1r_t[:], in_=b1r_d[:])
            nc.sync.dma_start(out=b2r_t[:], in_=b2r_d[:])
            id_t = cpool.tile([128, 128], f32)
            nc.sync.dma_start(out=id_t[:], in_=ident_d[:])
            par_t = cpool.tile([128, 1], f32)
            nc.sync.dma_start(out=par_t[:], in_=par_d[:])
            ad2st = stash.tile([128, BLOCKS_PER_CORE * T], f32)
            o2st = stash.tile([128, BLOCKS_PER_CORE * OUT_DIM], f32)

            # ---------------- Phase A (4 blocks per iteration) ----------------
            with tc.tile_pool(name="pa_x", bufs=3) as pax, \
                 tc.tile_pool(name="pa_ps", bufs=2, space="PSUM") as paps, \
                 tc.tile_pool(name="pa_row", bufs=3) as parow, \
                 tc.tile_pool(name="pa_ad", bufs=3) as paad:
                for n4 in range(NBLK // 4):
                    # one DMA: x for 4 blocks, both K halves -> [128, 4, 2, 128]
                    xt = pax.tile([128, 2, 4, 128], bf16, tag="xt")
                    xv = xTb[0:128, 0:128]
                    for k in range(2):
                        nc.sync.dma_start(
                            out=xt[:, k, :, :],
                            in_=bass.AP(xv.tensor, xv.offset + k * 128 * NTOT + n4 * 512,
                                        [[NTOT, 128], [128, 4], [1, 128]]))
                    row = parow.tile([128, 4, 130], f32, tag="row")
                    adsb = paad.tile([128, 4, 4], f32, tag="adsb")
                    for j in range(4):
                        ps = paps.tile([128, 264], f32, tag=f"ps{j}")
                        nc.tensor.matmul(out=ps[:], lhsT=xt[:, 0, j, :], rhs=w1e0[:],
                                         start=True, stop=False)
                        nc.tensor.matmul(out=ps[:], lhsT=xt[:, 1, j, :], rhs=w1e1[:],
                                         start=False, stop=True)
                        nc.vector.tensor_copy(out=row[:, j, 0:130].bitcast(bf16),
                                              in_=ps[:, 0:260])
                        nc.vector.tensor_copy(out=adsb[:, j, :], in_=ps[:, 260:264])
                    tab = tabL1_lo if n4 < NBLK // 8 else tabL1_hi
                    r0 = (n4 * 512) % HALF
                    tv = tab[:]
                    nc.sync.dma_start(
                        out=bass.AP(tv.tensor, tv.offset + r0 * ROW1,
                                    [[ROW1, 128], [ROW1 * 128, 4], [1, 130]]),
                        in_=row[:])
                    bv = blockad[:]
                    for j in range(4):
                        nc.sync.dma_start(
                            out=bass.AP(bv.tensor, bv.offset + (n4 * 4 + j) * 64 * 64,
                                        [[64, 64], [4, 2], [1, 4]]),
                            in_=adsb[:, j, :])

            if phases == "A":
                return nc

            # ---------------- L1 edge phase (+ fused layer-2 projection) ----
            l1n = BLOCKS_PER_CORE
            if phases.startswith("L1:"):
                l1n = int(phases.split(":")[1])
            with tc.tile_pool(name="g1", bufs=3) as g1p, \
                 tc.tile_pool(name="gidx", bufs=3) as gip, \
                 tc.tile_pool(name="sstk", bufs=3) as ssp, \
                 tc.tile_pool(name="gad", bufs=3) as gadp, \
                 tc.tile_pool(name="scr", bufs=3) as scrp, \
                 tc.tile_pool(name="post", bufs=3) as postp, \
                 tc.tile_pool(name="l1ps", bufs=2, space="PSUM") as l1ps, \
                 tc.tile_pool(name="tps", bufs=2, space="PSUM") as tps, \
                 tc.tile_pool(name="a2ps", bufs=2, space="PSUM") as a2ps, \
                 tc.tile_pool(name="adps", bufs=2, space="PSUM") as adpsp:
                for b in range(l1n):
                    ix = gip.tile([128, T * 8 + 8], i16, tag="ix")
                    nc.sync.dma_start(out=ix[:], in_=idx_all_d[b])
                    il = ix[:, 0:T_LO * 8]
                    ih = ix[:, T_LO * 8:T * 8]
                    ia = ix[:, T * 8:T * 8 + 8]
                    sks = ssp.tile([128, 2 * T * 128], fp8, tag="sks")
                    nc.sync.dma_start(out=sks[:], in_=stks_d[b])
                    sk = sks[:, 0:T * 128]
                    stk = sks[:, T * 128:2 * T * 128]

                    # block-ad gather first: the ad-matmul chain overlaps row gathers
                    gad = gadp.tile([128, 1, 64], f32, tag="gad")
                    nc.gpsimd.dma_gather(
                        out_ap=gad[:], in_ap=blockad[:], idxs_ap=ia,
                        num_idxs=128, num_idxs_reg=128, elem_size=64,
                        queue_num=3)
                    glo = g1p.tile([128, T_LO, ROW1], f32, tag="glo")
                    ghi = g1p.tile([128, T_HI, ROW1], f32, tag="ghi")
                    qn = 0
                    for g_t, tab, idxs, nt_s in ((glo, tabL1_lo, il, T_LO),
                                                 (ghi, tabL1_hi, ih, T_HI)):
                        for c0 in range(0, nt_s, 8):
                            cn = min(8, nt_s - c0)
                            nc.gpsimd.dma_gather(
                                out_ap=g_t[:, c0:c0 + cn, :], in_ap=tab[:],
                                idxs_ap=idxs[:, c0 * 8:(c0 + cn) * 8],
                                num_idxs=cn * 128, num_idxs_reg=cn * 128,
                                elem_size=ROW1, queue_num=qn % 3)
                            qn += 1

                    # block ad1 via parity select: ad = adA + par*(adB - adA)
                    dfa = scrp.tile([128, 4], f32, tag="dfa")
                    nc.vector.tensor_tensor(out=dfa[:], in0=gad[:, 0, 4:8],
                                            in1=gad[:, 0, 0:4], op=Alu.subtract)
                    nc.vector.tensor_tensor(
                        out=dfa[:], in0=dfa[:],
                        in1=ap(par_t[:], [[0, 4]]), op=Alu.mult)
                    adblk = scrp.tile([128, 4], bf16, tag="adblk")
                    nc.vector.tensor_tensor(out=adblk[:], in0=gad[:, 0, 0:4],
                                            in1=dfa[:], op=Alu.add)

                    # per-edge ad1: T matmuls lhsT=S_T fp8
                    adp = adpsp.tile([128, T * 4 + T], f32)
                    for t in range(T):
                        nc.tensor.matmul(out=adp[:, t * 4:(t + 1) * 4],
                                         lhsT=stk[:, t * 128:(t + 1) * 128],
                                         rhs=adblk[:], start=True, stop=True)

                    # logits = as + ad, lrelu, exp
                    pe = scrp.tile([128, T * 4], f32, tag="pe")
                    pev = pe[:].rearrange("p (t f) -> p t f", f=4)
                    adv_ = adp[:, 0:T * 4].rearrange("p (t f) -> p t f", f=4)
                    nc.vector.tensor_tensor(
                        out=pev[:, 0:T_LO, :], in0=adv_[:, 0:T_LO, :],
                        in1=ap(glo[:].bitcast(bf16), [[384, T_LO], [1, 4]], extra_off=256),
                        op=Alu.add)
                    nc.vector.tensor_tensor(
                        out=pev[:, T_LO:T, :], in0=adv_[:, T_LO:T, :],
                        in1=ap(ghi[:].bitcast(bf16), [[384, T_HI], [1, 4]], extra_off=256),
                        op=Alu.add)
                    u = scrp.tile([128, T * 4], f32, tag="u")
                    nc.vector.tensor_scalar_mul(out=u[:], in0=pe[:], scalar1=NEG)
                    nc.vector.tensor_tensor(out=pe[:], in0=pe[:], in1=u[:], op=Alu.max)
                    nc.scalar.activation(out=pe[:], in_=pe[:], func=Act.Exp)
                    # p -> bf16 into rows at word 128
                    nc.scalar.copy(out=glo[:, :, 128:130].bitcast(bf16),
                                   in_=pev[:, 0:T_LO, :])
                    nc.scalar.copy(out=ghi[:, :, 128:130].bitcast(bf16),
                                   in_=pev[:, T_LO:T, :])
                    # h *= p (per head), bf16
                    for g_t, nT in ((glo, T_LO), (ghi, T_HI)):
                        hb = g_t[:].bitcast(bf16)
                        nc.vector.tensor_tensor(
                            out=ap(hb, [[384, nT], [64, 4], [1, 64]]),
                            in0=ap(hb, [[384, nT], [64, 4], [1, 64]]),
                            in1=ap(hb, [[384, nT], [1, 4], [0, 64]], extra_off=256),
                            op=Alu.mult)

                    # scatter-add via one-hot matmuls
                    psb = l1ps.tile([128, 260], f32)
                    for t in range(T):
                        g_t, tt = (glo, t) if t < T_LO else (ghi, t - T_LO)
                        nc.tensor.matmul(out=psb[:],
                                         lhsT=sk[:, t * 128:(t + 1) * 128],
                                         rhs=g_t[:, tt, 0:130].bitcast(bf16),
                                         start=(t == 0), stop=(t == T - 1))
                    # divide + bias + ELU
                    dn = postp.tile([128, 4], f32, tag="dn")
                    nc.vector.tensor_scalar_add(out=dn[:], in0=psb[:, 256:260], scalar1=1e-16)
                    rcp = postp.tile([128, 4], f32, tag="rcp")
                    nc.vector.reciprocal(out=rcp[:], in_=dn[:])
                    o1 = postp.tile([128, 256], f32, tag="o1")
                    o1v = o1[:].rearrange("p (h c) -> p h c", h=4)
                    nc.vector.tensor_tensor(out=o1v, in0=psb[:, 0:256].rearrange("p (h c) -> p h c", h=4),
                                            in1=ap(rcp[:], [[1, 4], [0, 64]]), op=Alu.mult)
                    nc.vector.tensor_tensor(out=o1[:], in0=o1[:], in1=b1r_t[:], op=Alu.add)
                    em = postp.tile([128, 256], f32, tag="em")
                    nc.scalar.activation(out=em[:], in_=o1[:], func=Act.Relu, scale=-1.0)
                    nc.scalar.activation(out=em[:], in_=em[:], func=Act.Exp, scale=-1.0)
                    nc.vector.tensor_scalar_max(out=o1[:], in0=o1[:], scalar1=0.0)
                    nc.vector.tensor_tensor(out=o1[:], in0=o1[:], in1=em[:], op=Alu.add)
                    nc.vector.tensor_scalar_add(out=o1[:], in0=o1[:], scalar1=-1.0)
                    # layer-2 projection: h2 = o1 @ W2e (bf16)
                    ps2 = a2ps.tile([128, 42], f32)
                    for c_i, w2c in ((0, w2e0), (1, w2e1)):
                        pst = tps.tile([128, 128], f32)
                        nc.tensor.transpose(out=pst[:], in_=o1[:, c_i * 128:(c_i + 1) * 128],
                                            identity=id_t[:])
                        tsb = postp.tile([128, 128], bf16, tag=f"tsb{c_i}")
                        nc.scalar.copy(out=tsb[:], in_=pst[:])
                        nc.tensor.matmul(out=ps2[:], lhsT=tsb[:], rhs=w2c[:],
                                         start=(c_i == 0), stop=(c_i == 1))
                    h2row = postp.tile([128, 41], bf16, tag="h2row")
                    nc.scalar.copy(out=h2row[:], in_=ps2[:, 0:41])
                    nc.sync.dma_start(out=h2shard[b * 128:(b + 1) * 128, 0:41], in_=h2row[:])
                    # per-edge ad2 via S_T (still in SBUF), stash for L2
                    adblk2 = postp.tile([128, 1], bf16, tag="adblk2")
                    nc.scalar.copy(out=adblk2[:], in_=ps2[:, 41:42])
                    for t in range(T):
                        nc.tensor.matmul(out=adp[:, T * 4 + t:T * 4 + t + 1],
                                         lhsT=stk[:, t * 128:(t + 1) * 128],
                                         rhs=adblk2[:], start=True, stop=True)
                    nc.scalar.copy(out=ad2st[:, b * T:(b + 1) * T],
                                   in_=adp[:, T * 4:T * 4 + T])

            if phases == "A1" or phases.startswith("L1:"):
                return nc

            # ---------------- AllGather ----------------
            nc.gpsimd.collective_compute(
                "AllGather", mybir.AluOpType.bypass,
                replica_groups=[list(range(NC))],
                ins=[h2shard[:]], outs=[tabL2[:]])

            if phases == "A1C":
                return nc

            # ---------------- L2 edge phase ----------------
            with tc.tile_pool(name="g2", bufs=3) as g2p, \
                 tc.tile_pool(name="gidx2", bufs=3) as gip2, \
                 tc.tile_pool(name="sstk2", bufs=3) as ssp2, \
                 tc.tile_pool(name="scr2", bufs=3) as scrp2, \
                 tc.tile_pool(name="post2", bufs=3) as postp2, \
                 tc.tile_pool(name="l2ps", bufs=2, space="PSUM") as l2ps:
                for b in range(BLOCKS_PER_CORE):
                    ix = gip2.tile([128, T * 8], i16, tag="ix2")
                    nc.sync.dma_start(out=ix[:], in_=idx_all_d[b][:, 0:T * 8])
                    il = ix[:, 0:T_LO * 8]
                    ih = ix[:, T_LO * 8:T * 8]
                    sk = ssp2.tile([128, T * 128], fp8, tag="sk2")
                    nc.sync.dma_start(out=sk[:], in_=stks_d[b][:, 0:T * 128])

                    glo = g2p.tile([128, T_LO, ROW2], bf16, tag="glo2")
                    ghi = g2p.tile([128, T_HI, ROW2], bf16, tag="ghi2")
                    qn = 0
                    for g_t, r0, r1, idxs, nt_s in (
                            (glo, 0, HALF, il, T_LO),
                            (ghi, HALF, NTOT, ih, T_HI)):
                        for c0 in range(0, nt_s, 8):
                            cn = min(8, nt_s - c0)
                            nc.gpsimd.dma_gather(
                                out_ap=g_t[:, c0:c0 + cn, :],
                                in_ap=tabL2[r0:r1, :],
                                idxs_ap=idxs[:, c0 * 8:(c0 + cn) * 8],
                                num_idxs=cn * 128, num_idxs_reg=cn * 128,
                                elem_size=ROW2, queue_num=qn % 4)
                            qn += 1

                    pe = scrp2.tile([128, T], f32, tag="pe2")
                    pev = pe[:].rearrange("p (t f) -> p t f", f=1)
                    adv_ = ad2st[:, b * T:(b + 1) * T].rearrange("p (t f) -> p t f", f=1)
                    nc.vector.tensor_tensor(out=pev[:, 0:T_LO, :], in0=adv_[:, 0:T_LO, :],
                                            in1=glo[:, :, 40:41], op=Alu.add)
                    nc.vector.tensor_tensor(out=pev[:, T_LO:T, :], in0=adv_[:, T_LO:T, :],
                                            in1=ghi[:, :, 40:41], op=Alu.add)
                    u = scrp2.tile([128, T], f32, tag="u2")
                    nc.vector.tensor_scalar_mul(out=u[:], in0=pe[:], scalar1=NEG)
                    nc.vector.tensor_tensor(out=pe[:], in0=pe[:], in1=u[:], op=Alu.max)
                    nc.scalar.activation(out=pe[:], in_=pe[:], func=Act.Exp)
                    # h2 *= p2 ; write p2 into word 40
                    for g_t, tlo, nT in ((glo, 0, T_LO), (ghi, T_LO, T_HI)):
                        nc.vector.tensor_tensor(
                            out=ap(g_t[:], [[ROW2, nT], [1, 40]]),
                            in0=ap(g_t[:], [[ROW2, nT], [1, 40]]),
                            in1=ap(pe[:], [[1, nT], [0, 40]], extra_off=tlo),
                            op=Alu.mult)
                        nc.scalar.copy(out=g_t[:, :, 40:41],
                                       in_=pev[:, tlo:tlo + nT, :])
                    psb = l2ps.tile([128, 41], f32)
                    for t in range(T):
                        g_t, tt = (glo, t) if t < T_LO else (ghi, t - T_LO)
                        nc.tensor.matmul(out=psb[:],
                                         lhsT=sk[:, t * 128:(t + 1) * 128],
                                         rhs=g_t[:, tt, 0:41],
                                         start=(t == 0), stop=(t == T - 1))
                    dn = postp2.tile([128, 1], f32, tag="dn2")
                    nc.vector.tensor_scalar_add(out=dn[:], in0=psb[:, 40:41], scalar1=1e-16)
                    rcp = postp2.tile([128, 1], f32, tag="rcp2")
                    nc.vector.reciprocal(out=rcp[:], in_=dn[:])
                    nc.scalar.activation(out=o2st[:, b * OUT_DIM:(b + 1) * OUT_DIM],
                                         in_=psb[:, 0:40], func=Act.Copy,
                                         scale=rcp[:, 0:1])

                # batched log-softmax over all 49 blocks
                NB = BLOCKS_PER_CORE
                o2v = o2st[:].rearrange("p (b c) -> p b c", c=OUT_DIM)
                nc.vector.tensor_tensor(
                    out=o2v, in0=o2v,
                    in1=ap(b2r_t[:], [[0, NB], [1, OUT_DIM]]), op=Alu.add)
                mx = stash.tile([128, NB], f32)
                nc.vector.tensor_reduce(out=mx[:].rearrange("p (b f) -> p b f", f=1),
                                        in_=o2v, op=Alu.max, axis=mybir.AxisListType.X)
                sh = stash.tile([128, NB * OUT_DIM], f32)
                shv = sh[:].rearrange("p (b c) -> p b c", c=OUT_DIM)
                nc.vector.tensor_tensor(
                    out=shv, in0=o2v,
                    in1=ap(mx[:], [[1, NB], [0, OUT_DIM]]), op=Alu.subtract)
                ex = stash.tile([128, NB * OUT_DIM], f32)
                nc.scalar.activation(out=ex[:], in_=sh[:], func=Act.Exp)
                sm = stash.tile([128, NB], f32)
                nc.vector.tensor_reduce(out=sm[:].rearrange("p (b f) -> p b f", f=1),
                                        in_=ex[:].rearrange("p (b c) -> p b c", c=OUT_DIM),
                                        op=Alu.add, axis=mybir.AxisListType.X)
                lns = stash.tile([128, NB], f32)
                nc.scalar.activation(out=lns[:], in_=sm[:], func=Act.Ln)
                of = stash.tile([128, NB * OUT_DIM], f32)
                ofv = of[:].rearrange("p (b c) -> p b c", c=OUT_DIM)
                nc.vector.tensor_tensor(
                    out=ofv, in0=shv,
                    in1=ap(lns[:], [[1, NB], [0, OUT_DIM]]), op=Alu.subtract)
                ov = out_d[0:128, 0:OUT_DIM]
                nc.sync.dma_start(
                    out=bass.AP(ov.tensor, ov.offset,
                                [[OUT_DIM, 128], [OUT_DIM * 128, NB], [1, OUT_DIM]]),
                    in_=of[:])
    return nc


_CACHE = {}


LAST_EXEC_NS = -1


def kernel(**inputs):
    return _run(inputs, "full")


def _run(inputs, phases, trace=False, tmpdir=None):
    from concourse.bass_utils import run_bass_kernel_spmd
    shared, percore, (T_LO, T_HI), pos = _prep(inputs)
    key = (T_LO, T_HI, phases)
    if key not in _CACHE:
        nc = _build(T_LO, T_HI, phases)
        nc.compile()
        _CACHE[key] = nc
    nc = _CACHE[key]
    in_maps = []
    for c in range(NC):
        m = dict(shared)
        m.update(percore[c])
        in_maps.append(m)
    res = run_bass_kernel_spmd(nc, in_maps, list(range(NC)), trace=trace, tmpdir=tmpdir)
    global LAST_EXEC_NS
    if res.exec_time_ns is not None:
        LAST_EXEC_NS = res.exec_time_ns
    full = np.concatenate([res.results[c]["out"] for c in range(NC)], axis=0)
    return np.ascontiguousarray(full[pos]).astype(np.float32)
